# revision 1
# baseline (speedup 1.0000x reference)
"""Trainium2 Bass kernel for CBSA (cross-block self-attention) module.

Shapes (hardcoded from the problem spec):
  x: [8, 4096, 512], proj_w/to_out_w: [512, 512], step_rep/step_x: [8,1,1],
  to_out_b: [512].  Output: [8, 4096, 512].

Sharding: data-parallel over batch, 1 batch per NeuronCore (8 cores).
"""

import numpy as np
import ml_dtypes

import concourse.bass as bass
import concourse.tile as tile
from concourse import bacc, mybir
from concourse import bass_utils

F32 = mybir.dt.float32
F32R = mybir.dt.float32r
BF16 = mybir.dt.bfloat16
FP8 = mybir.dt.float8e4

B = 8
N = 4096
C = 512
HEADS = 8
DH = 64
Q = 64            # pooled tokens
SCALE = DH ** -0.5
NT = N // 128     # 32 token tiles
CH = C // 128     # 4 feature chunks
PAIRS = HEADS // 2  # 4 head pairs
NS = N // 512     # 8 free-dim slices of 512

_CACHE = {}
DEBUG = False


def _build():
    nc = bacc.Bacc("TRN2", target_bir_lowering=False, debug=False, num_devices=B)

    xT_d = nc.dram_tensor("xT", [128, 2, 2, N], FP8, kind="ExternalInput").ap()
    pwT_d = nc.dram_tensor("pwT", [128, 2, 2, C], FP8, kind="ExternalInput").ap()
    twT_d = nc.dram_tensor("twT", [128, CH, C], BF16, kind="ExternalInput").ap()
    bias_d = nc.dram_tensor("bias", [128, C], F32, kind="ExternalInput").ap()
    srep_d = nc.dram_tensor("srep", [128, PAIRS], F32, kind="ExternalInput").ap()
    idf_d = nc.dram_tensor("identf", [128, 128], F32, kind="ExternalInput").ap()
    idb_d = nc.dram_tensor("identb", [128, 128], BF16, kind="ExternalInput").ap()
    out_d = nc.dram_tensor("out", [N, C], F32, kind="ExternalOutput").ap()
    taps = {}
    if DEBUG:
        def tapdecl(name, shape, dt):
            taps[name] = nc.dram_tensor("tap_" + name, shape, dt,
                                        kind="ExternalOutput").ap()
        tapdecl("wtb0", [128, N], BF16)
        tapdecl("w0", [128, C], BF16)
        tapdecl("rep", [Q, C], F32)
        tapdecl("rep_pair", [128, PAIRS * DH], F32)
        tapdecl("dblk0", [128, 128], BF16)
        tapdecl("ed0", [128, N], BF16)
        tapdecl("s10", [128, 1], F32)
        tapdecl("at0", [128, 128], BF16)
        tapdecl("rd0", [128, 128], F32)
        tapdecl("rnat0", [128, 128], BF16)
        tapdecl("rnT0", [128, 128], BF16)
        tapdecl("ed20", [128, 128], BF16)
        tapdecl("xds0", [128, 128], BF16)
        tapdecl("xdT0", [128, N], BF16)

    from contextlib import ExitStack
    with tile.TileContext(nc) as tc:
        with ExitStack() as ctx:
            _body.ctx = ctx
            _body(tc, nc, xT_d, pwT_d, twT_d, bias_d, srep_d, idf_d, idb_d,
                  out_d, taps)
    nc.compile()
    return nc


def _body(tc, nc, xT_d, pwT_d, twT_d, bias_d, srep_d, idf_d, idb_d, out_d,
          taps=None):
    def tap(name, ap):
        if taps and name in taps:
            nc.sync.dma_start(taps[name][:], ap)
    Exp = mybir.ActivationFunctionType.Exp
    X = mybir.AxisListType.X
    mult = mybir.AluOpType.add  # placeholder; real ops referenced inline

    ctx = _body.ctx
    const = ctx.enter_context(tc.tile_pool(name="const", bufs=1))
    persist = ctx.enter_context(tc.tile_pool(name="persist", bufs=1))
    xs_pool = ctx.enter_context(tc.tile_pool(name="xstream", bufs=3))
    ed_pool = ctx.enter_context(tc.tile_pool(name="ed", bufs=4))
    m_pool = ctx.enter_context(tc.tile_pool(name="mpool", bufs=4))
    at_pool = ctx.enter_context(tc.tile_pool(name="at", bufs=3))
    sm_pool = ctx.enter_context(tc.tile_pool(name="small", bufs=2))
    ost_pool = ctx.enter_context(tc.tile_pool(name="ostage", bufs=2))
    ps512 = ctx.enter_context(tc.tile_pool(name="ps512", bufs=3, space="PSUM"))
    ps128 = ctx.enter_context(tc.tile_pool(name="ps128", bufs=2, space="PSUM"))
    pstr = ctx.enter_context(tc.tile_pool(name="pstr", bufs=3, space="PSUM"))

    # ---- constants / small inputs ----
    # Issue order matters: SP processes DMAs FIFO, so load what phase 1
    # needs first (pwT), defer late-phase constants (identf/srep/twT/bias).
    pwT = const.tile([128, 2, 2, C], FP8, tag="pwT")
    nc.sync.dma_start(pwT[:], pwT_d[:])
    # ---- persistent intermediates ----
    # wtb[di]: wT chunk di in bf16, [128 (d local), N]
    wtb = [persist.tile([128, N], BF16, tag=f"big{di}", name=f"wtb{di}")
           for di in range(CH)]
    # w natural, bf16: [128 (n local), NT, C]
    w_sb = persist.tile([128, NT, C], BF16, tag="w_sb")
    # x_deltaT chunks, bf16

    # ================= Phase 1: wT = proj_w @ x^T  =================
    # out[d, n] = sum_c proj_w[d, c] x[n, c]; lhsT = pwT[ci][:, di*128:...],
    # rhs = xT[ci][:, s*512:...] streamed from DRAM.
    for sl2 in range(NS // 2):
        xts = xs_pool.tile([128, 2, 2, 1024], FP8, tag="xs", name="xts")
        nc.sync.dma_start(xts[:], xT_d[:, :, :, sl2 * 1024:(sl2 + 1) * 1024])
        for di in range(CH):
            pst = [ps512.tile([128, 512], F32, tag="ps512", name=f"pst{j}")
                   for j in range(2)]
            for g in range(2):
                for s2 in range(2):
                    nc.tensor.matmul(
                        pst[s2][:],
                        pwT[:, g, :, di * 128:(di + 1) * 128],
                        xts[:, g, :, s2 * 512:(s2 + 1) * 512],
                        start=(g == 0), stop=(g == 1),
                        perf_mode=mybir.MatmulPerfMode.DoubleRow,
                    )
            for s2 in range(2):
                sl = sl2 * 2 + s2
                nc.scalar.activation(wtb[di][:, sl * 512:(sl + 1) * 512],
                                     pst[s2][:],
                                     mybir.ActivationFunctionType.Copy,
                                     scale=1.0 / 16.0)

    # deferred constants (needed from phase 2 onward)
    identb = const.tile([128, 128], BF16, tag="identb")
    nc.sync.dma_start(identb[:], idb_d[:])
    identf = const.tile([128, 128], F32, tag="identf")
    nc.sync.dma_start(identf[:], idf_d[:])
    srep = const.tile([128, PAIRS], F32, tag="srep")
    nc.sync.dma_start(srep[:], srep_d[:])
    twT = const.tile([128, CH, C], BF16, tag="twT")
    nc.sync.dma_start(twT[:], twT_d[:])
    bias = const.tile([128, C], F32, tag="bias")
    nc.sync.dma_start(bias[:], bias_d[:])

    # ================= Phase 2: w natural via PE transposes (batched) ====
    for di in range(CH):
        for t0 in range(0, NT, 8):
            wtp = pstr.tile([128, 8, 128], BF16, tag="pstr", name="wtp")
            for j in range(8):
                nc.tensor.transpose(wtp[:, j, :],
                                    wtb[di][:, (t0 + j) * 128:(t0 + j + 1) * 128],
                                    identb[:])
            if t0 % 16 == 0:
                nc.vector.tensor_copy(w_sb[:, t0:t0 + 8, di * 128:(di + 1) * 128],
                                      wtp[:])
            else:
                nc.scalar.copy(w_sb[:, t0:t0 + 8, di * 128:(di + 1) * 128], wtp[:])

    # ================= Phase 3: pooled rep via DVE strided reduce =========
    # repT[d, q] = sum over the 64 member tokens of each pooled cell.
    # n = qh*512 + i*64 + qw*8 + j  (qh,qw = cell; i,j = intra-cell)
    repT_sb = sm_pool.tile([128, CH, Q], F32, tag="repT_sb")
    pool_t = sm_pool.tile([128, 8, 8], F32, tag="pool_t")
    for di in range(CH):
        for qh in range(8):
            src = wtb[di][:, qh * 512:(qh + 1) * 512]
            # stage A: reduce contiguous j (8) -> [qw, i]
            nc.vector.tensor_reduce(
                pool_t[:], src.rearrange("p (i qw j) -> p qw i j", i=8, qw=8, j=8),
                mybir.AxisListType.X, mybir.AluOpType.add)
            # stage B: reduce contiguous i (8) -> [qw]
            nc.vector.tensor_reduce(
                repT_sb[:, di, qh * 8:(qh + 1) * 8], pool_t[:],
                mybir.AxisListType.X, mybir.AluOpType.add)
        nc.vector.tensor_scalar_mul(repT_sb[:, di, :], repT_sb[:, di, :],
                                    1.0 / 64.0)
    tap("repT", repT_sb.rearrange("p a b -> p (a b)")[:])

    # rep_pair[qp, p, dh]: rows 0:64 = head 2p queries, 64:128 = head 2p+1.
    # Transpose repT halves back to natural [q, dh] via identity matmuls.
    rep_pair = sm_pool.tile([128, PAIRS, DH], F32, tag="rep_pair")
    rp_ps = ps128.tile([128, PAIRS * DH], F32, tag="ps128")
    for p in range(PAIRS):
        nc.tensor.matmul(rp_ps[0:64, p * DH:(p + 1) * DH],
                         repT_sb[0:64, p, :], identf[0:64, 0:64],
                         start=True, stop=True)
        nc.tensor.matmul(rp_ps[64:128, p * DH:(p + 1) * DH],
                         repT_sb[64:128, p, :], identf[64:128, 64:128],
                         start=True, stop=True)
    nc.vector.tensor_copy(rep_pair.rearrange("p a b -> p (a b)")[:], rp_ps[:])
    tap("rep_pair", rep_pair.rearrange("p a b -> p (a b)")[:])

    # block-diag lhsT for dots (bf16)
    dblk = []
    for p in range(PAIRS):
        bk = sm_pool.tile([128, 128], BF16, tag=f"dblk{p}")
        nc.vector.memset(bk[:], 0.0)
        nc.vector.tensor_copy(bk[0:64, 0:64], repT_sb[0:64, p, :])
        nc.vector.tensor_copy(bk[64:128, 64:128], repT_sb[64:128, p, :])
        dblk.append(bk)

    tap("dblk0", dblk[0][:])
    # ================= Phase 4: per head-pair attention =================
    M_list = []
    ed_list = []
    for p in range(PAIRS):
        # --- dots + exp + row sums ---
        ed = ed_pool.tile([128, N], BF16, tag="ed")
        s1parts = sm_pool.tile([128, NS], F32, tag="s1parts")
        for s in range(NS):
            dps = ps512.tile([128, 512], F32, tag="ps512")
            nc.tensor.matmul(dps[:], dblk[p][:], wtb[p][:, s * 512:(s + 1) * 512],
                             start=True, stop=True)
            nc.scalar.activation(ed[:, s * 512:(s + 1) * 512], dps[:], Exp,
                                 scale=SCALE, accum_out=s1parts[:, s:s + 1])
        s1 = sm_pool.tile([128, 1], F32, tag="s1")
        nc.vector.tensor_reduce(s1[:], s1parts[:], X, mybir.AluOpType.add)
        rc1 = sm_pool.tile([128, 1], F32, tag="rc1")
        nc.vector.reciprocal(rc1[:], s1[:])
        ssc = sm_pool.tile([128, 1], F32, tag="ssc")
        nc.vector.tensor_mul(ssc[:], rc1[:], srep[:, p:p + 1])
        if p == 0:
            tap("ed0", ed[:])
            tap("s10", s1[:])

        # --- attnT via PE transposes (batched copies) ---
        at = at_pool.tile([128, NT, 128], BF16, tag="at")
        for t0 in range(0, NT, 8):
            atp = pstr.tile([128, 8, 128], BF16, tag="pstr", name="atp")
            for j in range(8):
                nc.tensor.transpose(atp[:, j, :],
                                    ed[:, (t0 + j) * 128:(t0 + j + 1) * 128],
                                    identb[:])
            if t0 % 16 == 0:
                nc.vector.tensor_copy(at[:, t0:t0 + 8, :], atp[:])
            else:
                nc.scalar.copy(at[:, t0:t0 + 8, :], atp[:])

        # --- rep_delta[qpair, d-block p] ---
        rd_ps = ps128.tile([128, 128], F32, tag="ps128")
        for t in range(NT):
            nc.tensor.matmul(rd_ps[:], at[:, t, :], w_sb[:, t, p * 128:(p + 1) * 128],
                             start=(t == 0), stop=(t == NT - 1))

        if p == 0:
            tap("at0", at[:, 0, :])
            rd_tap = sm_pool.tile([128, 128], F32, tag="rd_tap")
            nc.vector.tensor_copy(rd_tap[:], rd_ps[:])
            tap("rd0", rd_tap[:])
        # --- reph_new (block-diag, natural layout) ---
        rnat = sm_pool.tile([128, 128], BF16, tag="rnat")
        nc.vector.memset(rnat[:], 0.0)
        for h in range(2):
            r0, r1 = 64 * h, 64 * (h + 1)
            nc.vector.scalar_tensor_tensor(
                rnat[r0:r1, r0:r1], rd_ps[r0:r1, r0:r1], ssc[r0:r1, 0:1],
                rep_pair[r0:r1, p, :],
                mybir.AluOpType.mult, mybir.AluOpType.add)

        # --- reph_new^T via PE transpose ---
        rtp = pstr.tile([128, 128], BF16, tag="pstr", name="rtp")
        nc.tensor.transpose(rtp[:], rnat[:], identb[:])
        rnT = sm_pool.tile([128, 128], BF16, tag="rnT")
        nc.vector.tensor_copy(rnT[:], rtp[:])

        if p == 0:
            tap("rnat0", rnat[:])
            tap("rnT0", rnT[:])
        # --- dots2 (block-diag, symmetric) + exp + sums ---
        d2_ps = ps128.tile([128, 128], F32, tag="ps128")
        nc.tensor.matmul(d2_ps[:], rnT[:], rnT[:], start=True, stop=True)
        ed2 = sm_pool.tile([128, 128], BF16, tag="ed2")
        nc.vector.memset(ed2[:], 0.0)
        s2 = sm_pool.tile([128, 1], F32, tag="s2")
        for h in range(2):
            r0, r1 = 64 * h, 64 * (h + 1)
            nc.scalar.activation(ed2[r0:r1, r0:r1], d2_ps[r0:r1, r0:r1], Exp,
                                 scale=SCALE, accum_out=s2[r0:r1, 0:1])

        # --- xds = attn2 @ reph_new, then scale rows by 1/(s1*s2) ---
        xds_ps = ps128.tile([128, 128], F32, tag="ps128")
        nc.tensor.matmul(xds_ps[:], ed2[:], rnat[:], start=True, stop=True)
        rc2 = sm_pool.tile([128, 1], F32, tag="rc2")
        nc.vector.reciprocal(rc2[:], s2[:])
        sc = sm_pool.tile([128, 1], F32, tag="sc")
        nc.vector.tensor_mul(sc[:], rc1[:], rc2[:])
        xds = sm_pool.tile([128, 128], BF16, tag="xds")
        nc.vector.tensor_scalar_mul(xds[:], xds_ps[:], sc[:])

        if p == 0:
            tap("ed20", ed2[:])
            tap("xds0", xds[:])
        # --- M_pair = xds^T @ twT_pair  (folds to_out into pooled space) ---
        xtp = pstr.tile([128, 128], BF16, tag="pstr", name="xtp")
        nc.tensor.transpose(xtp[:], xds[:], identb[:])
        xdsT = sm_pool.tile([128, 128], BF16, tag="xdsT")
        nc.vector.tensor_copy(xdsT[:], xtp[:])
        mp_ps = ps512.tile([128, 512], F32, tag="ps512", name="mp_ps")
        nc.tensor.matmul(mp_ps[:], xdsT[:], twT[:, p, :], start=True, stop=True)
        M_p = m_pool.tile([128, C], BF16, tag="M_p", name=f"M_{p}")
        nc.vector.tensor_copy(M_p[:], mp_ps[:])
        M_list.append(M_p)
        ed_list.append(ed)

    # ================= Phase 5: out = sum_p ed_p^T @ M_p + bias =========
    for t0 in range(0, NT, 4):
        ot = ost_pool.tile([128, 4, C], F32, tag="ostage", name="ot")
        for j in range(4):
            t = t0 + j
            ops = ps512.tile([128, 512], F32, tag="ps512", name="ops")
            for p in range(PAIRS):
                nc.tensor.matmul(ops[:], ed_list[p][:, t * 128:(t + 1) * 128],
                                 M_list[p][:],
                                 start=(p == 0), stop=(p == PAIRS - 1))
            nc.vector.tensor_add(ot[:, j, :], ops[:], bias[:])
        nc.sync.dma_start(
            out_d.rearrange("(a b) c -> b a c", b=128)[:, t0:t0 + 4, :], ot[:])

def _prep_inputs(x, proj_w, step_rep, step_x, to_out_w, to_out_b):
    x = np.asarray(x, dtype=np.float32)
    proj_w = np.asarray(proj_w, dtype=np.float32)
    step_rep = np.asarray(step_rep, dtype=np.float32).reshape(HEADS)
    step_x = np.asarray(step_x, dtype=np.float32).reshape(HEADS)
    to_out_w = np.asarray(to_out_w, dtype=np.float32)
    to_out_b = np.asarray(to_out_b, dtype=np.float32)

    # fp8 DoubleRow packing: c = 256*g + 2*k + j
    pw16 = (proj_w.T * 16.0).reshape(2, 128, 2, C)
    pwT = np.ascontiguousarray(pw16.transpose(1, 0, 2, 3)).astype(
        ml_dtypes.float8_e4m3)
    twTs = np.ascontiguousarray(to_out_w.T) * np.repeat(step_x, DH)[:, None]
    twTs = np.ascontiguousarray(
        twTs.reshape(CH, 128, C).transpose(1, 0, 2)).astype(ml_dtypes.bfloat16)
    bias = np.broadcast_to(to_out_b, (128, C)).copy()

    srep = np.empty((128, PAIRS), dtype=np.float32)
    for p in range(PAIRS):
        srep[0:64, p] = step_rep[2 * p]
        srep[64:128, p] = step_rep[2 * p + 1]

    identf = np.eye(128, dtype=np.float32)
    identb = np.eye(128, dtype=ml_dtypes.bfloat16)

    shared = {
        "pwT": pwT, "twT": twTs, "bias": bias,
        "srep": srep, "identf": identf, "identb": identb,
    }
    in_maps = []
    for b in range(B):
        xT = np.ascontiguousarray(
            x[b].T.reshape(2, 128, 2, N).transpose(1, 0, 2, 3)).astype(
            ml_dtypes.float8_e4m3)
        in_maps.append({"xT": xT, **shared})
    return in_maps


def kernel(x, proj_w, step_rep, step_x, to_out_w, to_out_b):
    if "nc" not in _CACHE:
        _CACHE["nc"] = _build()
    nc = _CACHE["nc"]
    in_maps = _prep_inputs(x, proj_w, step_rep, step_x, to_out_w, to_out_b)
    res = bass_utils.run_bass_kernel_spmd(nc, in_maps, core_ids=list(range(B)))
    return np.stack([res.results[b]["out"] for b in range(B)], axis=0)



# revision 6
# speedup vs baseline: 1.1558x; 1.1558x over previous
"""Trainium2 Bass kernel for CBSA (cross-block self-attention) module.

Shapes (hardcoded from the problem spec):
  x: [8, 4096, 512], proj_w/to_out_w: [512, 512], step_rep/step_x: [8,1,1],
  to_out_b: [512].  Output: [8, 4096, 512].

Sharding: data-parallel over batch, 1 batch per NeuronCore (8 cores).

Structure (v2):
  - pooling is linear and commutes with the proj GEMM -> pooled x is
    computed on host, rep^T comes from a tiny on-device GEMM.
  - P1 streams x^T in 4 chunks; per chunk: wT GEMM slices, then dots,
    exp, w-transposes and attn-transposes interleaved so PE never idles.
  - per-pair tail: rep_delta, rep update, self-attn among pooled reps,
    M = xds @ to_out^T (folded).
  - P5 emits out TRANSPOSED [C, N] so the bias is a per-partition scalar
    fused into the PSUM->SBUF copy; output is bf16, host transposes back.
"""

from contextlib import ExitStack

import numpy as np
import ml_dtypes

import concourse.bass as bass
import concourse.tile as tile
from concourse import bacc, mybir
from concourse import bass_utils

F32 = mybir.dt.float32
BF16 = mybir.dt.bfloat16

B = 8
N = 4096
C = 512
HEADS = 8
DH = 64
Q = 64            # pooled tokens
SCALE = DH ** -0.5
NT = N // 128     # 32 token tiles
CH = C // 128     # 4 feature chunks
PAIRS = HEADS // 2  # 4 head pairs
NS = N // 512     # 8 free-dim slices of 512
NCHUNK = 4        # x streamed in 4 chunks of 1024 tokens

_CACHE = {}


def _build():
    nc = bacc.Bacc("TRN2", target_bir_lowering=False, debug=False, num_devices=B)

    xT_d = nc.dram_tensor("xT", [128, CH, N], BF16, kind="ExternalInput").ap()
    pwT_d = nc.dram_tensor("pwT", [128, CH, C], BF16, kind="ExternalInput").ap()
    xpT_d = nc.dram_tensor("xpT", [128, CH, Q], BF16, kind="ExternalInput").ap()
    twT_d = nc.dram_tensor("twT", [128, CH, C], BF16, kind="ExternalInput").ap()
    biasT_d = nc.dram_tensor("biasT", [128, CH], F32, kind="ExternalInput").ap()
    srep_d = nc.dram_tensor("srep", [128, PAIRS], F32, kind="ExternalInput").ap()
    idb_d = nc.dram_tensor("identb", [128, 128], BF16, kind="ExternalInput").ap()
    out_d = nc.dram_tensor("out", [C, N], BF16, kind="ExternalOutput").ap()

    with tile.TileContext(nc) as tc:
        with ExitStack() as ctx:
            _body.ctx = ctx
            _body(tc, nc, xT_d, pwT_d, xpT_d, twT_d, biasT_d, srep_d, idb_d,
                  out_d)
    nc.compile()
    return nc


def _body(tc, nc, xT_d, pwT_d, xpT_d, twT_d, biasT_d, srep_d, idb_d, out_d):
    Exp = mybir.ActivationFunctionType.Exp
    Ident = mybir.ActivationFunctionType.Identity
    X = mybir.AxisListType.X
    ADD = mybir.AluOpType.add
    MULT = mybir.AluOpType.mult

    ctx = _body.ctx
    const = ctx.enter_context(tc.tile_pool(name="const", bufs=1))
    persist = ctx.enter_context(tc.tile_pool(name="persist", bufs=1))
    xs_pool = ctx.enter_context(tc.tile_pool(name="xstream", bufs=2))
    sm_pool = ctx.enter_context(tc.tile_pool(name="small", bufs=2))
    ost_pool = ctx.enter_context(tc.tile_pool(name="ostage", bufs=2))

    # ---- constants / small inputs (issued before the big x stream) ----
    pwT = const.tile([128, CH, C], BF16, tag="pwT")
    nc.sync.dma_start(pwT[:], pwT_d[:])
    xpT = const.tile([128, CH, Q], BF16, tag="xpT")
    nc.sync.dma_start(xpT[:], xpT_d[:])
    identb = const.tile([128, 128], BF16, tag="identb")
    nc.sync.dma_start(identb[:], idb_d[:])
    srep = const.tile([128, PAIRS], F32, tag="srep")
    nc.sync.dma_start(srep[:], srep_d[:])
    twT = const.tile([128, CH, C], BF16, tag="twT")
    nc.sync.dma_start(twT[:], twT_d[:])
    biasT = const.tile([128, CH], F32, tag="biasT")
    nc.sync.dma_start(biasT[:], biasT_d[:])

    # ---- persistent intermediates ----
    wtb = [persist.tile([128, N], BF16, tag=f"wtb{di}", name=f"wtb{di}")
           for di in range(CH)]
    w_sb = persist.tile([128, NT, C], BF16, tag="w_sb")          # w natural
    ed_list = [persist.tile([128, N], BF16, tag=f"ed{p}", name=f"ed{p}")
               for p in range(PAIRS)]
    at_list = [persist.tile([128, NT, 128], BF16, tag=f"at{p}", name=f"at{p}")
               for p in range(PAIRS)]
    M_list = [persist.tile([128, C], BF16, tag=f"M{p}", name=f"M{p}")
              for p in range(PAIRS)]

    with ExitStack() as phase_ctx:
        psB = phase_ctx.enter_context(
            tc.tile_pool(name="psB", bufs=3, space="PSUM"))
        psC = phase_ctx.enter_context(
            tc.tile_pool(name="psC", bufs=2, space="PSUM"))
        psD = phase_ctx.enter_context(
            tc.tile_pool(name="psD", bufs=3, space="PSUM"))

        # ============ P0: repT via tiny GEMM from host-pooled x ==========
        # repT[d, q] = sum_c pw[d, c] * xp[q, c];  lhsT = pwT[c, d] chunks.
        dblk = []
        rpT = []
        for p in range(PAIRS):
            rps = psD.tile([128, Q], F32, tag="psD", name=f"rps{p}")
            for ci in range(CH):
                nc.tensor.matmul(rps[:], pwT[:, ci, p * 128:(p + 1) * 128],
                                 xpT[:, ci, :],
                                 start=(ci == 0), stop=(ci == CH - 1))
            bk = sm_pool.tile([128, 128], BF16, tag=f"dblk{p}", name=f"dblk{p}")
            nc.vector.memset(bk[:], 0.0)
            nc.vector.tensor_copy(bk[0:64, 0:64], rps[0:64, :])
            nc.vector.tensor_copy(bk[64:128, 64:128], rps[64:128, :])
            dblk.append(bk)
        # rep natural (block-diag) = transpose of dblk
        for p in range(PAIRS):
            rp_ps = psC.tile([128, 128], BF16, tag="psC", name=f"rpps{p}")
            nc.tensor.transpose(rp_ps[:], dblk[p][:], identb[:])
            rp = sm_pool.tile([128, 128], BF16, tag=f"rpT{p}", name=f"rpT{p}")
            nc.vector.tensor_copy(rp[:], rp_ps[:])
            rpT.append(rp)

        # ============ P1: streamed wT GEMM + dots + exp + transposes ======
        # wT[d, n] = sum_c pw[d, c] x[n, c]; lhsT = pwT chunk, rhs = xT chunk
        s1parts = [sm_pool.tile([128, NS], F32, tag=f"s1p{p}", name=f"s1p{p}")
                   for p in range(PAIRS)]
        for ck in range(NCHUNK):
            cols = N // NCHUNK                         # 1024
            c0 = ck * cols
            xts = xs_pool.tile([128, CH, cols], BF16, tag="xs", name="xts")
            nc.sync.dma_start(xts[:], xT_d[:, :, c0:c0 + cols])
            # wT slices for this chunk
            for di in range(CH):
                for s2 in range(2):
                    wps = psB.tile([128, 512], F32, tag="psB", name="wps")
                    for ci in range(CH):
                        nc.tensor.matmul(
                            wps[:], pwT[:, ci, di * 128:(di + 1) * 128],
                            xts[:, ci, s2 * 512:(s2 + 1) * 512],
                            start=(ci == 0), stop=(ci == CH - 1))
                    dst = wtb[di][:, c0 + s2 * 512:c0 + (s2 + 1) * 512]
                    if s2 == 0:
                        nc.vector.tensor_copy(dst, wps[:])
                    else:
                        nc.scalar.copy(dst, wps[:])
            # dots + exp for this chunk's two 512-slices, all pairs
            for p in range(PAIRS):
                for s2 in range(2):
                    sl = ck * 2 + s2
                    dps = psB.tile([128, 512], F32, tag="psB", name="dps")
                    nc.tensor.matmul(dps[:], dblk[p][:],
                                     wtb[p][:, sl * 512:(sl + 1) * 512],
                                     start=True, stop=True)
                    nc.scalar.activation(
                        ed_list[p][:, sl * 512:(sl + 1) * 512], dps[:], Exp,
                        scale=SCALE, accum_out=s1parts[p][:, sl:sl + 1])
            # w natural transposes for this chunk (8 token tiles)
            for t0 in range(ck * 8, ck * 8 + 8, 4):
                for di in range(CH):
                    wtp = psC.tile([128, 4, 128], BF16, tag="psC", name="wtp")
                    for j in range(4):
                        nc.tensor.transpose(
                            wtp[:, j, :],
                            wtb[di][:, (t0 + j) * 128:(t0 + j + 1) * 128],
                            identb[:])
                    if di % 2 == 0:
                        nc.vector.tensor_copy(
                            w_sb[:, t0:t0 + 4, di * 128:(di + 1) * 128],
                            wtp[:])
                    else:
                        nc.scalar.copy(
                            w_sb[:, t0:t0 + 4, di * 128:(di + 1) * 128],
                            wtp[:])
            # attn transposes for this chunk
            for p in range(PAIRS):
                for t0 in range(ck * 8, ck * 8 + 8, 4):
                    atp = psC.tile([128, 4, 128], BF16, tag="psC", name="atp")
                    for j in range(4):
                        nc.tensor.transpose(
                            atp[:, j, :],
                            ed_list[p][:, (t0 + j) * 128:(t0 + j + 1) * 128],
                            identb[:])
                    nc.vector.tensor_copy(at_list[p][:, t0:t0 + 4, :], atp[:])

        # ============ P4: per-pair pooled attention tail =================
        for p in range(PAIRS):
            s1 = sm_pool.tile([128, 1], F32, tag="s1")
            nc.vector.tensor_reduce(s1[:], s1parts[p][:], X, ADD)
            rc1 = sm_pool.tile([128, 1], F32, tag="rc1")
            nc.vector.reciprocal(rc1[:], s1[:])
            ssc = sm_pool.tile([128, 1], F32, tag="ssc")
            nc.vector.tensor_mul(ssc[:], rc1[:], srep[:, p:p + 1])

            # rep_delta[q, d-pair] = sum_n attn^T w
            rd_ps = psD.tile([128, 128], F32, tag="psD", name="rd_ps")
            for t in range(NT):
                nc.tensor.matmul(rd_ps[:], at_list[p][:, t, :],
                                 w_sb[:, t, p * 128:(p + 1) * 128],
                                 start=(t == 0), stop=(t == NT - 1))
            # reph_new (block-diag, natural layout)
            rnat = sm_pool.tile([128, 128], BF16, tag="rnat")
            nc.vector.memset(rnat[:], 0.0)
            for h in range(2):
                r0, r1 = 64 * h, 64 * (h + 1)
                nc.vector.scalar_tensor_tensor(
                    rnat[r0:r1, r0:r1], rd_ps[r0:r1, r0:r1], ssc[r0:r1, 0:1],
                    rpT[p][r0:r1, r0:r1], MULT, ADD)
            # reph_new^T
            rtp = psC.tile([128, 128], BF16, tag="psC", name="rtp")
            nc.tensor.transpose(rtp[:], rnat[:], identb[:])
            rnT = sm_pool.tile([128, 128], BF16, tag="rnT")
            nc.vector.tensor_copy(rnT[:], rtp[:])
            # dots2 (block-diag, symmetric) + exp + sums
            d2_ps = psD.tile([128, 128], F32, tag="psD", name="d2_ps")
            nc.tensor.matmul(d2_ps[:], rnT[:], rnT[:], start=True, stop=True)
            ed2 = sm_pool.tile([128, 128], BF16, tag="ed2")
            nc.vector.memset(ed2[:], 0.0)
            s2 = sm_pool.tile([128, 1], F32, tag="s2")
            for h in range(2):
                r0, r1 = 64 * h, 64 * (h + 1)
                nc.scalar.activation(ed2[r0:r1, r0:r1], d2_ps[r0:r1, r0:r1],
                                     Exp, scale=SCALE,
                                     accum_out=s2[r0:r1, 0:1])
            # xds = attn2 @ reph_new, scaled by 1/(s1*s2)
            xds_ps = psD.tile([128, 128], F32, tag="psD", name="xds_ps")
            nc.tensor.matmul(xds_ps[:], ed2[:], rnat[:], start=True, stop=True)
            rc2 = sm_pool.tile([128, 1], F32, tag="rc2")
            nc.vector.reciprocal(rc2[:], s2[:])
            sc = sm_pool.tile([128, 1], F32, tag="sc")
            nc.vector.tensor_mul(sc[:], rc1[:], rc2[:])
            xds = sm_pool.tile([128, 128], BF16, tag="xds")
            nc.vector.tensor_scalar_mul(xds[:], xds_ps[:], sc[:])
            # M = xds @ twT_pair (folds to_out + step_x into pooled space)
            xtp = psC.tile([128, 128], BF16, tag="psC", name="xtp")
            nc.tensor.transpose(xtp[:], xds[:], identb[:])
            xdsT = sm_pool.tile([128, 128], BF16, tag="xdsT")
            nc.vector.tensor_copy(xdsT[:], xtp[:])
            mp_ps = psB.tile([128, 512], F32, tag="psB", name="mp_ps")
            nc.tensor.matmul(mp_ps[:], xdsT[:], twT[:, p, :],
                             start=True, stop=True)
            nc.vector.tensor_copy(M_list[p][:], mp_ps[:])

    # ============ P5: outT[d, n] = sum_p M_p^T @ ed_p  (+bias) ==========
    with tc.tile_pool(name="psA", bufs=3, space="PSUM") as psA:
        for dc in range(CH):
            osb = ost_pool.tile([128, N], BF16, tag="osb", name="osb")
            for g in range(NS // 2):
                ops = psA.tile([128, 2, 512], F32, tag="psA", name="ops")
                for sli in range(2):
                    sl = g * 2 + sli
                    for p in range(PAIRS):
                        nc.tensor.matmul(
                            ops[:, sli, :],
                            M_list[p][:, dc * 128:(dc + 1) * 128],
                            ed_list[p][:, sl * 512:(sl + 1) * 512],
                            start=(p == 0), stop=(p == PAIRS - 1))
                dst = osb[:, g * 1024:(g + 1) * 1024]
                src = ops.rearrange("p a b -> p (a b)")
                if g % 2 == 0:
                    nc.scalar.activation(dst, src[:], Ident,
                                         bias=biasT[:, dc:dc + 1], scale=1.0)
                else:
                    nc.vector.tensor_scalar(dst, src[:], 1.0,
                                            biasT[:, dc:dc + 1], MULT, ADD)
            nc.sync.dma_start(out_d[dc * 128:(dc + 1) * 128, :], osb[:])


def _prep_inputs(x, proj_w, step_rep, step_x, to_out_w, to_out_b):
    x = np.asarray(x, dtype=np.float32)
    proj_w = np.asarray(proj_w, dtype=np.float32)
    step_rep = np.asarray(step_rep, dtype=np.float32).reshape(HEADS)
    step_x = np.asarray(step_x, dtype=np.float32).reshape(HEADS)
    to_out_w = np.asarray(to_out_w, dtype=np.float32)
    to_out_b = np.asarray(to_out_b, dtype=np.float32)

    pwT = np.ascontiguousarray(
        proj_w.T.reshape(CH, 128, C).transpose(1, 0, 2)).astype(
        ml_dtypes.bfloat16)
    twTs = np.ascontiguousarray(to_out_w.T) * np.repeat(step_x, DH)[:, None]
    twTs = np.ascontiguousarray(
        twTs.reshape(CH, 128, C).transpose(1, 0, 2)).astype(ml_dtypes.bfloat16)
    biasT = np.ascontiguousarray(
        to_out_b.reshape(CH, 128).T).astype(np.float32)

    srep = np.empty((128, PAIRS), dtype=np.float32)
    for p in range(PAIRS):
        srep[0:64, p] = step_rep[2 * p]
        srep[64:128, p] = step_rep[2 * p + 1]

    identb = np.eye(128, dtype=ml_dtypes.bfloat16)

    shared = {
        "pwT": pwT, "twT": twTs, "biasT": biasT,
        "srep": srep, "identb": identb,
    }
    in_maps = []
    for b in range(B):
        xT = np.ascontiguousarray(
            x[b].T.reshape(CH, 128, N).transpose(1, 0, 2)).astype(
            ml_dtypes.bfloat16)
        # pooled x: spatial 64x64 grid, 8x8 block means -> [64, C]
        xp = x[b].reshape(8, 8, 8, 8, C).mean(axis=(1, 3)).reshape(Q, C)
        xpT = np.ascontiguousarray(
            xp.T.reshape(CH, 128, Q).transpose(1, 0, 2)).astype(
            ml_dtypes.bfloat16)
        in_maps.append({"xT": xT, "xpT": xpT, **shared})
    return in_maps


def kernel(x, proj_w, step_rep, step_x, to_out_w, to_out_b):
    if "nc" not in _CACHE:
        _CACHE["nc"] = _build()
    nc = _CACHE["nc"]
    in_maps = _prep_inputs(x, proj_w, step_rep, step_x, to_out_w, to_out_b)
    res = bass_utils.run_bass_kernel_spmd(nc, in_maps, core_ids=list(range(B)))
    return np.stack(
        [np.asarray(res.results[b]["out"]).astype(np.float32).T
         for b in range(B)], axis=0)


# revision 7
# speedup vs baseline: 1.3043x; 1.1285x over previous
"""Trainium2 Bass kernel for CBSA (cross-block self-attention) module.

Shapes (hardcoded from the problem spec):
  x: [8, 4096, 512], proj_w/to_out_w: [512, 512], step_rep/step_x: [8,1,1],
  to_out_b: [512].  Output: [8, 4096, 512].

Sharding: data-parallel over batch, 1 batch per NeuronCore (8 cores).

Structure (v2):
  - pooling is linear and commutes with the proj GEMM -> pooled x is
    computed on host, rep^T comes from a tiny on-device GEMM.
  - P1 streams x^T in 4 chunks; per chunk: wT GEMM slices, then dots,
    exp, w-transposes and attn-transposes interleaved so PE never idles.
  - per-pair tail: rep_delta, rep update, self-attn among pooled reps,
    M = xds @ to_out^T (folded).
  - P5 emits out TRANSPOSED [C, N] so the bias is a per-partition scalar
    fused into the PSUM->SBUF copy; output is bf16, host transposes back.
"""

from contextlib import ExitStack

import numpy as np
import ml_dtypes

import concourse.bass as bass
import concourse.tile as tile
from concourse import bacc, mybir
from concourse import bass_utils

F32 = mybir.dt.float32
BF16 = mybir.dt.bfloat16
FP8 = mybir.dt.float8e4

B = 8
N = 4096
C = 512
HEADS = 8
DH = 64
Q = 64            # pooled tokens
SCALE = DH ** -0.5
NT = N // 128     # 32 token tiles
CH = C // 128     # 4 feature chunks
PAIRS = HEADS // 2  # 4 head pairs
NS = N // 512     # 8 free-dim slices of 512
NCHUNK = 4        # x streamed in 4 chunks of 1024 tokens

_CACHE = {}


def _build():
    nc = bacc.Bacc("TRN2", target_bir_lowering=False, debug=False, num_devices=B)

    xT_d = nc.dram_tensor("xT", [128, 2, 2, N], FP8, kind="ExternalInput").ap()
    pwT8_d = nc.dram_tensor("pwT8", [128, 2, 2, C], FP8, kind="ExternalInput").ap()
    pwT_d = nc.dram_tensor("pwT", [128, CH, C], BF16, kind="ExternalInput").ap()
    xpT_d = nc.dram_tensor("xpT", [128, CH, Q], BF16, kind="ExternalInput").ap()
    twT_d = nc.dram_tensor("twT", [128, CH, C], BF16, kind="ExternalInput").ap()
    biasT_d = nc.dram_tensor("biasT", [128, CH], F32, kind="ExternalInput").ap()
    srep_d = nc.dram_tensor("srep", [128, PAIRS], F32, kind="ExternalInput").ap()
    idb_d = nc.dram_tensor("identb", [128, 128], BF16, kind="ExternalInput").ap()
    out_d = nc.dram_tensor("out", [C, N], BF16, kind="ExternalOutput").ap()

    with tile.TileContext(nc) as tc:
        with ExitStack() as ctx:
            _body.ctx = ctx
            _body(tc, nc, xT_d, pwT8_d, pwT_d, xpT_d, twT_d, biasT_d, srep_d,
                  idb_d, out_d)
    nc.compile()
    return nc


def _body(tc, nc, xT_d, pwT8_d, pwT_d, xpT_d, twT_d, biasT_d, srep_d, idb_d,
          out_d):
    Exp = mybir.ActivationFunctionType.Exp
    Ident = mybir.ActivationFunctionType.Identity
    X = mybir.AxisListType.X
    ADD = mybir.AluOpType.add
    MULT = mybir.AluOpType.mult

    ctx = _body.ctx
    const = ctx.enter_context(tc.tile_pool(name="const", bufs=1))
    persist = ctx.enter_context(tc.tile_pool(name="persist", bufs=1))
    xs_pool = ctx.enter_context(tc.tile_pool(name="xstream", bufs=2))
    sm_pool = ctx.enter_context(tc.tile_pool(name="small", bufs=2))
    ost_pool = ctx.enter_context(tc.tile_pool(name="ostage", bufs=2))

    # ---- constants / small inputs (issued before the big x stream) ----
    pwT8 = const.tile([128, 2, 2, C], FP8, tag="pwT8")
    nc.sync.dma_start(pwT8[:], pwT8_d[:])
    pwT = const.tile([128, CH, C], BF16, tag="pwT")
    nc.sync.dma_start(pwT[:], pwT_d[:])
    xpT = const.tile([128, CH, Q], BF16, tag="xpT")
    nc.sync.dma_start(xpT[:], xpT_d[:])
    # prefetch first two x chunks ahead of the remaining constants
    cols = N // NCHUNK
    xts_list = []
    for ck in range(NCHUNK):
        xts = xs_pool.tile([128, 2, 2, cols], FP8, tag="xs", name=f"xts{ck}")
        xts_list.append(xts)
    for ck in range(2):
        nc.sync.dma_start(xts_list[ck][:],
                          xT_d[:, :, :, ck * cols:(ck + 1) * cols])
    identb = const.tile([128, 128], BF16, tag="identb")
    nc.sync.dma_start(identb[:], idb_d[:])
    srep = const.tile([128, PAIRS], F32, tag="srep")
    nc.sync.dma_start(srep[:], srep_d[:])
    twT = const.tile([128, CH, C], BF16, tag="twT")
    nc.sync.dma_start(twT[:], twT_d[:])
    biasT = const.tile([128, CH], F32, tag="biasT")
    nc.sync.dma_start(biasT[:], biasT_d[:])

    # ---- persistent intermediates ----
    wtb = [persist.tile([128, N], BF16, tag=f"wtb{di}", name=f"wtb{di}")
           for di in range(CH)]
    w_sb = persist.tile([128, NT, C], BF16, tag="w_sb")          # w natural
    ed_list = [persist.tile([128, N], BF16, tag=f"ed{p}", name=f"ed{p}")
               for p in range(PAIRS)]
    at_list = [persist.tile([128, NT, 128], BF16, tag=f"at{p}", name=f"at{p}")
               for p in range(PAIRS)]
    M_list = [persist.tile([128, C], BF16, tag=f"M{p}", name=f"M{p}")
              for p in range(PAIRS)]

    with ExitStack() as phase_ctx:
        psB = phase_ctx.enter_context(
            tc.tile_pool(name="psB", bufs=3, space="PSUM"))
        psC = phase_ctx.enter_context(
            tc.tile_pool(name="psC", bufs=2, space="PSUM"))
        psD = phase_ctx.enter_context(
            tc.tile_pool(name="psD", bufs=3, space="PSUM"))

        # ============ P0: repT via tiny GEMM from host-pooled x ==========
        # repT[d, q] = sum_c pw[d, c] * xp[q, c];  lhsT = pwT[c, d] chunks.
        dblk = []
        rpT = []
        for p in range(PAIRS):
            rps = psD.tile([128, Q], F32, tag="psD", name=f"rps{p}")
            for ci in range(CH):
                nc.tensor.matmul(rps[:], pwT[:, ci, p * 128:(p + 1) * 128],
                                 xpT[:, ci, :],
                                 start=(ci == 0), stop=(ci == CH - 1))
            bk = sm_pool.tile([128, 128], BF16, tag=f"dblk{p}", name=f"dblk{p}")
            nc.vector.memset(bk[:], 0.0)
            nc.vector.tensor_copy(bk[0:64, 0:64], rps[0:64, :])
            nc.vector.tensor_copy(bk[64:128, 64:128], rps[64:128, :])
            dblk.append(bk)
        # rep natural (block-diag) = transpose of dblk
        for p in range(PAIRS):
            rp_ps = psC.tile([128, 128], BF16, tag="psC", name=f"rpps{p}")
            nc.tensor.transpose(rp_ps[:], dblk[p][:], identb[:])
            rp = sm_pool.tile([128, 128], BF16, tag=f"rpT{p}", name=f"rpT{p}")
            nc.vector.tensor_copy(rp[:], rp_ps[:])
            rpT.append(rp)

        # ============ P1: streamed wT GEMM + dots + exp + transposes ======
        # wT[d, n] = sum_c pw[d, c] x[n, c]; lhsT = pwT chunk, rhs = xT chunk
        s1parts = [sm_pool.tile([128, NS], F32, tag=f"s1p{p}", name=f"s1p{p}")
                   for p in range(PAIRS)]
        Copy = mybir.ActivationFunctionType.Copy
        for ck in range(NCHUNK):
            c0 = ck * cols
            xts = xts_list[ck]
            if ck >= 2:
                nc.sync.dma_start(xts[:], xT_d[:, :, :, c0:c0 + cols])
            # wT slices for this chunk (fp8 DoubleRow)
            for di in range(CH):
                for s2 in range(2):
                    wps = psB.tile([128, 512], F32, tag="psB", name="wps")
                    for g in range(2):
                        nc.tensor.matmul(
                            wps[:], pwT8[:, g, :, di * 128:(di + 1) * 128],
                            xts[:, g, :, s2 * 512:(s2 + 1) * 512],
                            start=(g == 0), stop=(g == 1),
                            perf_mode=mybir.MatmulPerfMode.DoubleRow)
                    dst = wtb[di][:, c0 + s2 * 512:c0 + (s2 + 1) * 512]
                    if s2 == 0:
                        nc.vector.tensor_scalar_mul(dst, wps[:], 1.0 / 16.0)
                    else:
                        nc.scalar.activation(dst, wps[:], Copy,
                                             scale=1.0 / 16.0)
            # dots + exp for this chunk's two 512-slices, all pairs
            for p in range(PAIRS):
                for s2 in range(2):
                    sl = ck * 2 + s2
                    dps = psB.tile([128, 512], F32, tag="psB", name="dps")
                    nc.tensor.matmul(dps[:], dblk[p][:],
                                     wtb[p][:, sl * 512:(sl + 1) * 512],
                                     start=True, stop=True)
                    nc.scalar.activation(
                        ed_list[p][:, sl * 512:(sl + 1) * 512], dps[:], Exp,
                        scale=SCALE, accum_out=s1parts[p][:, sl:sl + 1])
            # w natural transposes for this chunk (8 token tiles)
            for t0 in range(ck * 8, ck * 8 + 8, 4):
                for di in range(CH):
                    wtp = psC.tile([128, 4, 128], BF16, tag="psC", name="wtp")
                    for j in range(4):
                        nc.tensor.transpose(
                            wtp[:, j, :],
                            wtb[di][:, (t0 + j) * 128:(t0 + j + 1) * 128],
                            identb[:])
                    if di % 2 == 0:
                        nc.vector.tensor_copy(
                            w_sb[:, t0:t0 + 4, di * 128:(di + 1) * 128],
                            wtp[:])
                    else:
                        nc.scalar.copy(
                            w_sb[:, t0:t0 + 4, di * 128:(di + 1) * 128],
                            wtp[:])
            # attn transposes for this chunk
            for p in range(PAIRS):
                for t0 in range(ck * 8, ck * 8 + 8, 4):
                    atp = psC.tile([128, 4, 128], BF16, tag="psC", name="atp")
                    for j in range(4):
                        nc.tensor.transpose(
                            atp[:, j, :],
                            ed_list[p][:, (t0 + j) * 128:(t0 + j + 1) * 128],
                            identb[:])
                    nc.vector.tensor_copy(at_list[p][:, t0:t0 + 4, :], atp[:])

        # ============ P4: per-pair pooled attention tail =================
        for p in range(PAIRS):
            s1 = sm_pool.tile([128, 1], F32, tag="s1")
            nc.vector.tensor_reduce(s1[:], s1parts[p][:], X, ADD)
            rc1 = sm_pool.tile([128, 1], F32, tag="rc1")
            nc.vector.reciprocal(rc1[:], s1[:])
            ssc = sm_pool.tile([128, 1], F32, tag="ssc")
            nc.vector.tensor_mul(ssc[:], rc1[:], srep[:, p:p + 1])

            # rep_delta[q, d-pair] = sum_n attn^T w
            rd_ps = psD.tile([128, 128], F32, tag="psD", name="rd_ps")
            for t in range(NT):
                nc.tensor.matmul(rd_ps[:], at_list[p][:, t, :],
                                 w_sb[:, t, p * 128:(p + 1) * 128],
                                 start=(t == 0), stop=(t == NT - 1))
            # reph_new (block-diag, natural layout)
            rnat = sm_pool.tile([128, 128], BF16, tag="rnat")
            nc.vector.memset(rnat[:], 0.0)
            for h in range(2):
                r0, r1 = 64 * h, 64 * (h + 1)
                nc.vector.scalar_tensor_tensor(
                    rnat[r0:r1, r0:r1], rd_ps[r0:r1, r0:r1], ssc[r0:r1, 0:1],
                    rpT[p][r0:r1, r0:r1], MULT, ADD)
            # reph_new^T
            rtp = psC.tile([128, 128], BF16, tag="psC", name="rtp")
            nc.tensor.transpose(rtp[:], rnat[:], identb[:])
            rnT = sm_pool.tile([128, 128], BF16, tag="rnT")
            nc.vector.tensor_copy(rnT[:], rtp[:])
            # dots2 (block-diag, symmetric) + exp + sums
            d2_ps = psD.tile([128, 128], F32, tag="psD", name="d2_ps")
            nc.tensor.matmul(d2_ps[:], rnT[:], rnT[:], start=True, stop=True)
            ed2 = sm_pool.tile([128, 128], BF16, tag="ed2")
            nc.vector.memset(ed2[:], 0.0)
            s2 = sm_pool.tile([128, 1], F32, tag="s2")
            for h in range(2):
                r0, r1 = 64 * h, 64 * (h + 1)
                nc.scalar.activation(ed2[r0:r1, r0:r1], d2_ps[r0:r1, r0:r1],
                                     Exp, scale=SCALE,
                                     accum_out=s2[r0:r1, 0:1])
            # xds = attn2 @ reph_new, scaled by 1/(s1*s2)
            xds_ps = psD.tile([128, 128], F32, tag="psD", name="xds_ps")
            nc.tensor.matmul(xds_ps[:], ed2[:], rnat[:], start=True, stop=True)
            rc2 = sm_pool.tile([128, 1], F32, tag="rc2")
            nc.vector.reciprocal(rc2[:], s2[:])
            sc = sm_pool.tile([128, 1], F32, tag="sc")
            nc.vector.tensor_mul(sc[:], rc1[:], rc2[:])
            xds = sm_pool.tile([128, 128], BF16, tag="xds")
            nc.vector.tensor_scalar_mul(xds[:], xds_ps[:], sc[:])
            # M = xds @ twT_pair (folds to_out + step_x into pooled space)
            xtp = psC.tile([128, 128], BF16, tag="psC", name="xtp")
            nc.tensor.transpose(xtp[:], xds[:], identb[:])
            xdsT = sm_pool.tile([128, 128], BF16, tag="xdsT")
            nc.vector.tensor_copy(xdsT[:], xtp[:])
            mp_ps = psB.tile([128, 512], F32, tag="psB", name="mp_ps")
            nc.tensor.matmul(mp_ps[:], xdsT[:], twT[:, p, :],
                             start=True, stop=True)
            nc.vector.tensor_copy(M_list[p][:], mp_ps[:])

    # ============ P5: outT[d, n] = sum_p M_p^T @ ed_p  (+bias) ==========
    with tc.tile_pool(name="psA", bufs=3, space="PSUM") as psA:
        for dc in range(CH):
            osb = ost_pool.tile([128, N], BF16, tag="osb", name="osb")
            for g in range(NS // 2):
                ops = psA.tile([128, 2, 512], F32, tag="psA", name="ops")
                for sli in range(2):
                    sl = g * 2 + sli
                    for p in range(PAIRS):
                        nc.tensor.matmul(
                            ops[:, sli, :],
                            M_list[p][:, dc * 128:(dc + 1) * 128],
                            ed_list[p][:, sl * 512:(sl + 1) * 512],
                            start=(p == 0), stop=(p == PAIRS - 1))
                dst = osb[:, g * 1024:(g + 1) * 1024]
                src = ops.rearrange("p a b -> p (a b)")
                if g % 2 == 0:
                    nc.scalar.activation(dst, src[:], Ident,
                                         bias=biasT[:, dc:dc + 1], scale=1.0)
                else:
                    nc.vector.tensor_scalar(dst, src[:], 1.0,
                                            biasT[:, dc:dc + 1], MULT, ADD)
            nc.sync.dma_start(out_d[dc * 128:(dc + 1) * 128, :], osb[:])


def _prep_inputs(x, proj_w, step_rep, step_x, to_out_w, to_out_b):
    x = np.asarray(x, dtype=np.float32)
    proj_w = np.asarray(proj_w, dtype=np.float32)
    step_rep = np.asarray(step_rep, dtype=np.float32).reshape(HEADS)
    step_x = np.asarray(step_x, dtype=np.float32).reshape(HEADS)
    to_out_w = np.asarray(to_out_w, dtype=np.float32)
    to_out_b = np.asarray(to_out_b, dtype=np.float32)

    pwT = np.ascontiguousarray(
        proj_w.T.reshape(CH, 128, C).transpose(1, 0, 2)).astype(
        ml_dtypes.bfloat16)
    pw16 = (proj_w.T * 16.0).reshape(2, 128, 2, C)
    pwT8 = np.ascontiguousarray(pw16.transpose(1, 0, 2, 3)).astype(
        ml_dtypes.float8_e4m3)
    twTs = np.ascontiguousarray(to_out_w.T) * np.repeat(step_x, DH)[:, None]
    twTs = np.ascontiguousarray(
        twTs.reshape(CH, 128, C).transpose(1, 0, 2)).astype(ml_dtypes.bfloat16)
    biasT = np.ascontiguousarray(
        to_out_b.reshape(CH, 128).T).astype(np.float32)

    srep = np.empty((128, PAIRS), dtype=np.float32)
    for p in range(PAIRS):
        srep[0:64, p] = step_rep[2 * p]
        srep[64:128, p] = step_rep[2 * p + 1]

    identb = np.eye(128, dtype=ml_dtypes.bfloat16)

    shared = {
        "pwT": pwT, "pwT8": pwT8, "twT": twTs, "biasT": biasT,
        "srep": srep, "identb": identb,
    }
    in_maps = []
    for b in range(B):
        xT = np.ascontiguousarray(
            x[b].T.reshape(2, 128, 2, N).transpose(1, 0, 2, 3)).astype(
            ml_dtypes.float8_e4m3)
        # pooled x: spatial 64x64 grid, 8x8 block means -> [64, C]
        xp = x[b].reshape(8, 8, 8, 8, C).mean(axis=(1, 3)).reshape(Q, C)
        xpT = np.ascontiguousarray(
            xp.T.reshape(CH, 128, Q).transpose(1, 0, 2)).astype(
            ml_dtypes.bfloat16)
        in_maps.append({"xT": xT, "xpT": xpT, **shared})
    return in_maps


def kernel(x, proj_w, step_rep, step_x, to_out_w, to_out_b):
    if "nc" not in _CACHE:
        _CACHE["nc"] = _build()
    nc = _CACHE["nc"]
    in_maps = _prep_inputs(x, proj_w, step_rep, step_x, to_out_w, to_out_b)
    res = bass_utils.run_bass_kernel_spmd(nc, in_maps, core_ids=list(range(B)))
    return np.stack(
        [np.asarray(res.results[b]["out"]).astype(np.float32).T
         for b in range(B)], axis=0)


# revision 9
# speedup vs baseline: 1.4165x; 1.0860x over previous
"""Trainium2 Bass kernel for CBSA (cross-block self-attention) module.

Shapes (hardcoded from the problem spec):
  x: [8, 4096, 512], proj_w/to_out_w: [512, 512], step_rep/step_x: [8,1,1],
  to_out_b: [512].  Output: [8, 4096, 512].

Sharding: data-parallel over batch, 1 batch per NeuronCore (8 cores).

Structure (v2):
  - pooling is linear and commutes with the proj GEMM -> pooled x is
    computed on host, rep^T comes from a tiny on-device GEMM.
  - P1 streams x^T in 4 chunks; per chunk: wT GEMM slices, then dots,
    exp, w-transposes and attn-transposes interleaved so PE never idles.
  - per-pair tail: rep_delta, rep update, self-attn among pooled reps,
    M = xds @ to_out^T (folded).
  - P5 emits out TRANSPOSED [C, N] so the bias is a per-partition scalar
    fused into the PSUM->SBUF copy; output is bf16, host transposes back.
"""

from contextlib import ExitStack

import numpy as np
import ml_dtypes

import concourse.bass as bass
import concourse.tile as tile
from concourse import bacc, mybir
from concourse import bass_utils

F32 = mybir.dt.float32
BF16 = mybir.dt.bfloat16
FP8 = mybir.dt.float8e4

B = 8
N = 4096
C = 512
HEADS = 8
DH = 64
Q = 64            # pooled tokens
SCALE = DH ** -0.5
NT = N // 128     # 32 token tiles
CH = C // 128     # 4 feature chunks
PAIRS = HEADS // 2  # 4 head pairs
NS = N // 512     # 8 free-dim slices of 512
NCHUNK = 4        # x streamed in 4 chunks of 1024 tokens

_CACHE = {}


def _build():
    nc = bacc.Bacc("TRN2", target_bir_lowering=False, debug=False, num_devices=B)

    xT_d = nc.dram_tensor("xT", [128, 2, 2, N], FP8, kind="ExternalInput").ap()
    pwT8_d = nc.dram_tensor("pwT8", [128, 2, 2, C], FP8, kind="ExternalInput").ap()
    pwT_d = nc.dram_tensor("pwT", [128, CH, C], BF16, kind="ExternalInput").ap()
    xpT_d = nc.dram_tensor("xpT", [128, CH, Q], BF16, kind="ExternalInput").ap()
    twT_d = nc.dram_tensor("twT", [128, CH, C], BF16, kind="ExternalInput").ap()
    biasT_d = nc.dram_tensor("biasT", [128, CH], F32, kind="ExternalInput").ap()
    srep_d = nc.dram_tensor("srep", [128, PAIRS], F32, kind="ExternalInput").ap()
    idb_d = nc.dram_tensor("identb", [128, 128], BF16, kind="ExternalInput").ap()
    id8_d = nc.dram_tensor("ident8", [128, 128], FP8, kind="ExternalInput").ap()
    out_d = nc.dram_tensor("out", [C, N], BF16, kind="ExternalOutput").ap()

    with tile.TileContext(nc) as tc:
        with ExitStack() as ctx:
            _body.ctx = ctx
            _body(tc, nc, xT_d, pwT8_d, pwT_d, xpT_d, twT_d, biasT_d, srep_d,
                  idb_d, id8_d, out_d)
    nc.compile()
    return nc


def _body(tc, nc, xT_d, pwT8_d, pwT_d, xpT_d, twT_d, biasT_d, srep_d, idb_d,
          id8_d, out_d):
    Exp = mybir.ActivationFunctionType.Exp
    Ident = mybir.ActivationFunctionType.Identity
    X = mybir.AxisListType.X
    ADD = mybir.AluOpType.add
    MULT = mybir.AluOpType.mult

    ctx = _body.ctx
    const = ctx.enter_context(tc.tile_pool(name="const", bufs=1))
    persist = ctx.enter_context(tc.tile_pool(name="persist", bufs=1))
    xs_pool = ctx.enter_context(tc.tile_pool(name="xstream", bufs=2))
    sm_pool = ctx.enter_context(tc.tile_pool(name="small", bufs=2))
    ost_pool = ctx.enter_context(tc.tile_pool(name="ostage", bufs=2))

    # ---- constants / small inputs (issued before the big x stream) ----
    pwT8 = const.tile([128, 2, 2, C], FP8, tag="pwT8")
    nc.sync.dma_start(pwT8[:], pwT8_d[:])
    pwT = const.tile([128, CH, C], BF16, tag="pwT")
    nc.sync.dma_start(pwT[:], pwT_d[:])
    xpT = const.tile([128, CH, Q], BF16, tag="xpT")
    nc.sync.dma_start(xpT[:], xpT_d[:])
    # prefetch first two x chunks ahead of the remaining constants
    cols = N // NCHUNK
    xts_list = []
    for ck in range(NCHUNK):
        xts = xs_pool.tile([128, 2, 2, cols], FP8, tag="xs", name=f"xts{ck}")
        xts_list.append(xts)
    for ck in range(2):
        nc.sync.dma_start(xts_list[ck][:],
                          xT_d[:, :, :, ck * cols:(ck + 1) * cols])
    identb = const.tile([128, 128], BF16, tag="identb")
    nc.sync.dma_start(identb[:], idb_d[:])
    ident8 = const.tile([128, 128], FP8, tag="ident8")
    nc.sync.dma_start(ident8[:], id8_d[:])
    srep = const.tile([128, PAIRS], F32, tag="srep")
    nc.sync.dma_start(srep[:], srep_d[:])
    twT = const.tile([128, CH, C], BF16, tag="twT")
    nc.sync.dma_start(twT[:], twT_d[:])
    biasT = const.tile([128, CH], F32, tag="biasT")
    nc.sync.dma_start(biasT[:], biasT_d[:])

    # ---- persistent intermediates ----
    wtb = [persist.tile([128, N], FP8, tag=f"wtb{di}", name=f"wtb{di}")
           for di in range(CH)]
    w_sb = persist.tile([128, NT, C], FP8, tag="w_sb")           # w natural
    ed_grp = [persist.tile([128, 2, N], FP8, tag=f"edg{u}", name=f"edg{u}")
              for u in range(PAIRS // 2)]
    at_list = [persist.tile([128, NT, 128], FP8, tag=f"at{p}", name=f"at{p}")
               for p in range(PAIRS)]
    M_grp = [persist.tile([128, 2, C], FP8, tag=f"Mg{u}", name=f"Mg{u}")
             for u in range(PAIRS // 2)]

    with ExitStack() as phase_ctx:
        psB = phase_ctx.enter_context(
            tc.tile_pool(name="psB", bufs=3, space="PSUM"))
        psC = phase_ctx.enter_context(
            tc.tile_pool(name="psC", bufs=2, space="PSUM"))
        psD = phase_ctx.enter_context(
            tc.tile_pool(name="psD", bufs=3, space="PSUM"))

        # ============ P0: repT via tiny GEMM from host-pooled x ==========
        # repT[d, q] = sum_c pw[d, c] * xp[q, c];  lhsT = pwT[c, d] chunks.
        dblk = []
        rpT = []
        for p in range(PAIRS):
            rps = psD.tile([128, Q], F32, tag="psD", name=f"rps{p}")
            for ci in range(CH):
                nc.tensor.matmul(rps[:], pwT[:, ci, p * 128:(p + 1) * 128],
                                 xpT[:, ci, :],
                                 start=(ci == 0), stop=(ci == CH - 1))
            bk = sm_pool.tile([128, 128], FP8, tag=f"dblk{p}", name=f"dblk{p}")
            nc.vector.memset(bk[:], 0.0)
            nc.vector.tensor_scalar_mul(bk[0:64, 0:64], rps[0:64, :], 16.0)
            nc.vector.tensor_scalar_mul(bk[64:128, 64:128], rps[64:128, :],
                                        16.0)
            dblk.append(bk)
        # rep natural (block-diag) = transpose of dblk
        for p in range(PAIRS):
            rp_ps = psC.tile([128, 256], FP8, tag="psC", name=f"rpps{p}")
            nc.tensor.transpose(rp_ps[:, 0:256:2], dblk[p][:], ident8[:])
            rp = sm_pool.tile([128, 128], BF16, tag=f"rpT{p}", name=f"rpT{p}")
            nc.vector.tensor_scalar_mul(rp[:], rp_ps[:, 0:256:2], 1.0 / 16.0)
            rpT.append(rp)

        # ============ P1: streamed wT GEMM + dots + exp + transposes ======
        # wT[d, n] = sum_c pw[d, c] x[n, c]; lhsT = pwT chunk, rhs = xT chunk
        s1parts = [sm_pool.tile([128, NS], F32, tag=f"s1p{p}", name=f"s1p{p}")
                   for p in range(PAIRS)]
        Copy = mybir.ActivationFunctionType.Copy
        for ck in range(NCHUNK):
            c0 = ck * cols
            xts = xts_list[ck]
            if ck >= 2:
                nc.sync.dma_start(xts[:], xT_d[:, :, :, c0:c0 + cols])
            # wT slices for this chunk (fp8 DoubleRow)
            for di in range(CH):
                for s2 in range(2):
                    wps = psB.tile([128, 512], F32, tag="psB", name="wps")
                    for g in range(2):
                        nc.tensor.matmul(
                            wps[:], pwT8[:, g, :, di * 128:(di + 1) * 128],
                            xts[:, g, :, s2 * 512:(s2 + 1) * 512],
                            start=(g == 0), stop=(g == 1),
                            perf_mode=mybir.MatmulPerfMode.DoubleRow)
                    dst = wtb[di][:, c0 + s2 * 512:c0 + (s2 + 1) * 512]
                    if s2 == 0:
                        nc.vector.tensor_scalar_mul(dst, wps[:], 1.0 / 16.0)
                    else:
                        nc.scalar.activation(dst, wps[:], Copy,
                                             scale=1.0 / 16.0)
            # dots + exp for this chunk's two 512-slices, all pairs
            for p in range(PAIRS):
                for s2 in range(2):
                    sl = ck * 2 + s2
                    dps = psB.tile([128, 512], F32, tag="psB", name="dps")
                    nc.tensor.matmul(dps[:], dblk[p][:],
                                     wtb[p][:, sl * 512:(sl + 1) * 512],
                                     start=True, stop=True)
                    nc.scalar.activation(
                        ed_grp[p // 2][:, p % 2, sl * 512:(sl + 1) * 512],
                        dps[:], Exp,
                        scale=SCALE / 16.0,
                        accum_out=s1parts[p][:, sl:sl + 1])
            # w natural transposes for this chunk (8 token tiles)
            for t0 in range(ck * 8, ck * 8 + 8, 4):
                for di in range(CH):
                    wtp = psC.tile([128, 4, 256], FP8, tag="psC", name="wtp")
                    for j in range(4):
                        nc.tensor.transpose(
                            wtp[:, j, 0:256:2],
                            wtb[di][:, (t0 + j) * 128:(t0 + j + 1) * 128],
                            ident8[:])
                    if di % 2 == 0:
                        nc.vector.tensor_copy(
                            w_sb[:, t0:t0 + 4, di * 128:(di + 1) * 128],
                            wtp[:, :, 0:256:2])
                    else:
                        nc.scalar.copy(
                            w_sb[:, t0:t0 + 4, di * 128:(di + 1) * 128],
                            wtp[:, :, 0:256:2])
            # attn transposes for this chunk
            for p in range(PAIRS):
                for t0 in range(ck * 8, ck * 8 + 8, 4):
                    atp = psC.tile([128, 4, 256], FP8, tag="psC", name="atp")
                    for j in range(4):
                        nc.tensor.transpose(
                            atp[:, j, 0:256:2],
                            ed_grp[p // 2][:, p % 2,
                                           (t0 + j) * 128:(t0 + j + 1) * 128],
                            ident8[:])
                    nc.vector.tensor_copy(at_list[p][:, t0:t0 + 4, :],
                                          atp[:, :, 0:256:2])

        # ============ P4: per-pair pooled attention tail =================
        for p in range(PAIRS):
            s1 = sm_pool.tile([128, 1], F32, tag="s1")
            nc.vector.tensor_reduce(s1[:], s1parts[p][:], X, ADD)
            rc1 = sm_pool.tile([128, 1], F32, tag="rc1")
            nc.vector.reciprocal(rc1[:], s1[:])
            ssc = sm_pool.tile([128, 1], F32, tag="ssc")
            nc.vector.tensor_mul(ssc[:], rc1[:], srep[:, p:p + 1])

            # rep_delta[q, d-pair] = sum_n attn^T w
            rd_ps = psD.tile([128, 128], F32, tag="psD", name="rd_ps")
            for t in range(NT):
                nc.tensor.matmul(rd_ps[:], at_list[p][:, t, :],
                                 w_sb[:, t, p * 128:(p + 1) * 128],
                                 start=(t == 0), stop=(t == NT - 1))
            # reph_new (block-diag, natural layout)
            rnat = sm_pool.tile([128, 128], BF16, tag="rnat")
            nc.vector.memset(rnat[:], 0.0)
            for h in range(2):
                r0, r1 = 64 * h, 64 * (h + 1)
                nc.vector.scalar_tensor_tensor(
                    rnat[r0:r1, r0:r1], rd_ps[r0:r1, r0:r1], ssc[r0:r1, 0:1],
                    rpT[p][r0:r1, r0:r1], MULT, ADD)
            # reph_new^T
            rtp = psC.tile([128, 128], BF16, tag="psC", name="rtp")
            nc.tensor.transpose(rtp[:], rnat[:], identb[:])
            rnT = sm_pool.tile([128, 128], BF16, tag="rnT")
            nc.vector.tensor_copy(rnT[:], rtp[:])
            # dots2 (block-diag, symmetric) + exp + sums
            d2_ps = psD.tile([128, 128], F32, tag="psD", name="d2_ps")
            nc.tensor.matmul(d2_ps[:], rnT[:], rnT[:], start=True, stop=True)
            ed2 = sm_pool.tile([128, 128], BF16, tag="ed2")
            nc.vector.memset(ed2[:], 0.0)
            s2 = sm_pool.tile([128, 1], F32, tag="s2")
            for h in range(2):
                r0, r1 = 64 * h, 64 * (h + 1)
                nc.scalar.activation(ed2[r0:r1, r0:r1], d2_ps[r0:r1, r0:r1],
                                     Exp, scale=SCALE,
                                     accum_out=s2[r0:r1, 0:1])
            # xds = attn2 @ reph_new, scaled by 1/(s1*s2)
            xds_ps = psD.tile([128, 128], F32, tag="psD", name="xds_ps")
            nc.tensor.matmul(xds_ps[:], ed2[:], rnat[:], start=True, stop=True)
            rc2 = sm_pool.tile([128, 1], F32, tag="rc2")
            nc.vector.reciprocal(rc2[:], s2[:])
            sc = sm_pool.tile([128, 1], F32, tag="sc")
            nc.vector.tensor_mul(sc[:], rc1[:], rc2[:])
            xds = sm_pool.tile([128, 128], BF16, tag="xds")
            nc.vector.tensor_scalar_mul(xds[:], xds_ps[:], sc[:])
            # M = xds @ twT_pair (folds to_out + step_x into pooled space)
            xtp = psC.tile([128, 128], BF16, tag="psC", name="xtp")
            nc.tensor.transpose(xtp[:], xds[:], identb[:])
            xdsT = sm_pool.tile([128, 128], BF16, tag="xdsT")
            nc.vector.tensor_copy(xdsT[:], xtp[:])
            mp_ps = psB.tile([128, 512], F32, tag="psB", name="mp_ps")
            nc.tensor.matmul(mp_ps[:], xdsT[:], twT[:, p, :],
                             start=True, stop=True)
            nc.vector.tensor_scalar_mul(M_grp[p // 2][:, p % 2, :], mp_ps[:],
                                        16.0)

    # ============ P5: outT[d, n] = sum_p M_p^T @ ed_p  (+bias) ==========
    with tc.tile_pool(name="psA", bufs=3, space="PSUM") as psA:
        for dc in range(CH):
            osb = ost_pool.tile([128, N], BF16, tag="osb", name="osb")
            for g in range(NS // 2):
                ops = psA.tile([128, 2, 512], F32, tag="psA", name="ops")
                for sli in range(2):
                    sl = g * 2 + sli
                    for u in range(PAIRS // 2):
                        nc.tensor.matmul(
                            ops[:, sli, :],
                            M_grp[u][:, :, dc * 128:(dc + 1) * 128],
                            ed_grp[u][:, :, sl * 512:(sl + 1) * 512],
                            start=(u == 0), stop=(u == PAIRS // 2 - 1),
                            perf_mode=mybir.MatmulPerfMode.DoubleRow)
                dst = osb[:, g * 1024:(g + 1) * 1024]
                osrc = ops.rearrange("p a b -> p (a b)")
                if g % 2 == 0:
                    nc.scalar.activation(dst, osrc[:], Ident,
                                         bias=biasT[:, dc:dc + 1],
                                         scale=1.0 / 16.0)
                else:
                    nc.vector.tensor_scalar(dst, osrc[:], 1.0 / 16.0,
                                            biasT[:, dc:dc + 1], MULT, ADD)
            nc.sync.dma_start(out_d[dc * 128:(dc + 1) * 128, :], osb[:])


def _prep_inputs(x, proj_w, step_rep, step_x, to_out_w, to_out_b):
    x = np.asarray(x, dtype=np.float32)
    proj_w = np.asarray(proj_w, dtype=np.float32)
    step_rep = np.asarray(step_rep, dtype=np.float32).reshape(HEADS)
    step_x = np.asarray(step_x, dtype=np.float32).reshape(HEADS)
    to_out_w = np.asarray(to_out_w, dtype=np.float32)
    to_out_b = np.asarray(to_out_b, dtype=np.float32)

    pwT = np.ascontiguousarray(
        proj_w.T.reshape(CH, 128, C).transpose(1, 0, 2)).astype(
        ml_dtypes.bfloat16)
    pw16 = (proj_w.T * 16.0).reshape(2, 128, 2, C)
    pwT8 = np.ascontiguousarray(pw16.transpose(1, 0, 2, 3)).astype(
        ml_dtypes.float8_e4m3)
    twTs = np.ascontiguousarray(to_out_w.T) * np.repeat(step_x, DH)[:, None]
    twTs = np.ascontiguousarray(
        twTs.reshape(CH, 128, C).transpose(1, 0, 2)).astype(ml_dtypes.bfloat16)
    biasT = np.ascontiguousarray(
        to_out_b.reshape(CH, 128).T).astype(np.float32)

    srep = np.empty((128, PAIRS), dtype=np.float32)
    for p in range(PAIRS):
        srep[0:64, p] = step_rep[2 * p]
        srep[64:128, p] = step_rep[2 * p + 1]

    identb = np.eye(128, dtype=ml_dtypes.bfloat16)
    ident8 = np.eye(128, dtype=ml_dtypes.float8_e4m3)

    shared = {
        "pwT": pwT, "pwT8": pwT8, "twT": twTs, "biasT": biasT,
        "srep": srep, "identb": identb, "ident8": ident8,
    }
    in_maps = []
    for b in range(B):
        xT = np.ascontiguousarray(
            x[b].T.reshape(2, 128, 2, N).transpose(1, 0, 2, 3)).astype(
            ml_dtypes.float8_e4m3)
        # pooled x: spatial 64x64 grid, 8x8 block means -> [64, C]
        xp = x[b].reshape(8, 8, 8, 8, C).mean(axis=(1, 3)).reshape(Q, C)
        xpT = np.ascontiguousarray(
            xp.T.reshape(CH, 128, Q).transpose(1, 0, 2)).astype(
            ml_dtypes.bfloat16)
        in_maps.append({"xT": xT, "xpT": xpT, **shared})
    return in_maps


def kernel(x, proj_w, step_rep, step_x, to_out_w, to_out_b):
    if "nc" not in _CACHE:
        _CACHE["nc"] = _build()
    nc = _CACHE["nc"]
    in_maps = _prep_inputs(x, proj_w, step_rep, step_x, to_out_w, to_out_b)
    res = bass_utils.run_bass_kernel_spmd(nc, in_maps, core_ids=list(range(B)))
    return np.stack(
        [np.asarray(res.results[b]["out"]).astype(np.float32).T
         for b in range(B)], axis=0)


# revision 13
# speedup vs baseline: 1.6983x; 1.1990x over previous
"""Trainium2 Bass kernel for CBSA (cross-block self-attention) module.

Shapes (hardcoded from the problem spec):
  x: [8, 4096, 512], proj_w/to_out_w: [512, 512], step_rep/step_x: [8,1,1],
  to_out_b: [512].  Output: [8, 4096, 512].

Sharding: data-parallel over batch, 1 batch per NeuronCore (8 cores).

Structure (v2):
  - pooling is linear and commutes with the proj GEMM -> pooled x is
    computed on host, rep^T comes from a tiny on-device GEMM.
  - P1 streams x^T in 4 chunks; per chunk: wT GEMM slices, then dots,
    exp, w-transposes and attn-transposes interleaved so PE never idles.
  - per-pair tail: rep_delta, rep update, self-attn among pooled reps,
    M = xds @ to_out^T (folded).
  - P5 emits out TRANSPOSED [C, N] so the bias is a per-partition scalar
    fused into the PSUM->SBUF copy; output is bf16, host transposes back.
"""

from contextlib import ExitStack

import numpy as np
import ml_dtypes

import concourse.bass as bass
import concourse.tile as tile
from concourse import bacc, mybir
from concourse import bass_utils

F32 = mybir.dt.float32
BF16 = mybir.dt.bfloat16
FP8 = mybir.dt.float8e4

B = 8
N = 4096
C = 512
HEADS = 8
DH = 64
Q = 64            # pooled tokens
SCALE = DH ** -0.5
NT = N // 128     # 32 token tiles
CH = C // 128     # 4 feature chunks
PAIRS = HEADS // 2  # 4 head pairs
NS = N // 512     # 8 free-dim slices of 512
NCHUNK = 8        # x streamed in 8 chunks of 512 tokens

_CACHE = {}


def _build():
    nc = bacc.Bacc("TRN2", target_bir_lowering=False, debug=False, num_devices=B)

    xT_d = nc.dram_tensor("xT", [128, 2, 2, N], FP8, kind="ExternalInput").ap()
    pwT8_d = nc.dram_tensor("pwT8", [128, 2, 2, C], FP8, kind="ExternalInput").ap()
    xpT_d = nc.dram_tensor("xpT", [128, 2, 2, Q], FP8, kind="ExternalInput").ap()
    twT_d = nc.dram_tensor("twT", [128, CH, C], BF16, kind="ExternalInput").ap()
    biasT_d = nc.dram_tensor("biasT", [128, CH], F32, kind="ExternalInput").ap()
    srep_d = nc.dram_tensor("srep", [128, PAIRS], F32, kind="ExternalInput").ap()
    idb_d = nc.dram_tensor("identb", [128, 128], BF16, kind="ExternalInput").ap()
    id8_d = nc.dram_tensor("ident8", [128, 128], FP8, kind="ExternalInput").ap()
    out_d = nc.dram_tensor("out", [C, N], BF16, kind="ExternalOutput").ap()

    with tile.TileContext(nc) as tc:
        with ExitStack() as ctx:
            _body.ctx = ctx
            _body(tc, nc, xT_d, pwT8_d, xpT_d, twT_d, biasT_d, srep_d,
                  idb_d, id8_d, out_d)
    nc.compile()
    return nc


def _body(tc, nc, xT_d, pwT8_d, xpT_d, twT_d, biasT_d, srep_d, idb_d,
          id8_d, out_d):
    Exp = mybir.ActivationFunctionType.Exp
    Ident = mybir.ActivationFunctionType.Identity
    X = mybir.AxisListType.X
    ADD = mybir.AluOpType.add
    MULT = mybir.AluOpType.mult

    ctx = _body.ctx
    const = ctx.enter_context(tc.tile_pool(name="const", bufs=1))
    persist = ctx.enter_context(tc.tile_pool(name="persist", bufs=1))
    xs_pool = ctx.enter_context(tc.tile_pool(name="xstream", bufs=2))
    sm_pool = ctx.enter_context(tc.tile_pool(name="small", bufs=2))
    ost_pool = ctx.enter_context(tc.tile_pool(name="ostage", bufs=2))

    # ---- constants / small inputs (issued before the big x stream) ----
    pwT8 = const.tile([128, 2, 2, C], FP8, tag="pwT8")
    nc.sync.dma_start(pwT8[:], pwT8_d[:])
    xpT = const.tile([128, 2, 2, Q], FP8, tag="xpT")
    nc.sync.dma_start(xpT[:], xpT_d[:])
    # prefetch first two x chunks ahead of the remaining constants
    cols = N // NCHUNK
    xts_list = []
    for ck in range(NCHUNK):
        xts = xs_pool.tile([128, 2, 2, cols], FP8, tag="xs", name=f"xts{ck}")
        xts_list.append(xts)
    for ck in range(3):
        nc.sync.dma_start(xts_list[ck][:],
                          xT_d[:, :, :, ck * cols:(ck + 1) * cols])
    identb = const.tile([128, 128], BF16, tag="identb")
    nc.sync.dma_start(identb[:], idb_d[:])
    ident8 = const.tile([128, 128], FP8, tag="ident8")
    nc.sync.dma_start(ident8[:], id8_d[:])
    srep = const.tile([128, PAIRS], F32, tag="srep")
    nc.sync.dma_start(srep[:], srep_d[:])
    twT = const.tile([128, CH, C], BF16, tag="twT")
    nc.sync.dma_start(twT[:], twT_d[:])
    biasT = const.tile([128, CH], F32, tag="biasT")
    nc.sync.dma_start(biasT[:], biasT_d[:])

    # ---- persistent intermediates ----
    wtb = [persist.tile([128, N], FP8, tag=f"wtb{di}", name=f"wtb{di}")
           for di in range(CH)]
    w_sb = persist.tile([128, NT, C], FP8, tag="w_sb")           # w natural
    ed_grp = [persist.tile([128, 2, N], FP8, tag=f"edg{u}", name=f"edg{u}")
              for u in range(PAIRS // 2)]
    at_list = [persist.tile([128, NT, 128], FP8, tag=f"at{p}", name=f"at{p}")
               for p in range(PAIRS)]
    M_grp = [persist.tile([128, 2, C], FP8, tag=f"Mg{u}", name=f"Mg{u}")
             for u in range(PAIRS // 2)]

    with ExitStack() as phase_ctx:
        psB = phase_ctx.enter_context(
            tc.tile_pool(name="psB", bufs=3, space="PSUM"))
        psC = phase_ctx.enter_context(
            tc.tile_pool(name="psC", bufs=2, space="PSUM"))
        psD = phase_ctx.enter_context(
            tc.tile_pool(name="psD", bufs=3, space="PSUM"))

        # ============ P0: repT via tiny fp8 GEMM from host-pooled x ======
        # repT[d, q] = sum_c pw[d, c] * xp[q, c]; psum = 256 * repT.
        dblk = []
        rpT = []
        for p in range(PAIRS):
            rps = psD.tile([128, Q], F32, tag="psD", name=f"rps{p}")
            for g in range(2):
                nc.tensor.matmul(rps[:], pwT8[:, g, :, p * 128:(p + 1) * 128],
                                 xpT[:, g, :, :],
                                 start=(g == 0), stop=(g == 1),
                                 perf_mode=mybir.MatmulPerfMode.DoubleRow)
            bk = sm_pool.tile([128, 128], FP8, tag=f"dblk{p}", name=f"dblk{p}")
            nc.vector.memset(bk[:], 0.0)
            nc.vector.tensor_scalar_mul(bk[0:64, 0:64], rps[0:64, :],
                                        1.0 / 16.0)
            nc.vector.tensor_scalar_mul(bk[64:128, 64:128], rps[64:128, :],
                                        1.0 / 16.0)
            dblk.append(bk)
        # rep natural (block-diag) = transpose of dblk
        for p in range(PAIRS):
            rp_ps = psC.tile([128, 256], FP8, tag="psC", name=f"rpps{p}")
            nc.tensor.transpose(rp_ps[:, 0:256:2], dblk[p][:], ident8[:])
            rp = sm_pool.tile([128, 128], BF16, tag=f"rpT{p}", name=f"rpT{p}")
            nc.vector.tensor_scalar_mul(rp[:], rp_ps[:, 0:256:2], 1.0 / 16.0)
            rpT.append(rp)

        # ============ P1: streamed wT GEMM + dots + exp + transposes ======
        # wT[d, n] = sum_c pw[d, c] x[n, c]; lhsT = pwT chunk, rhs = xT chunk
        s1parts = [sm_pool.tile([128, NS], F32, tag=f"s1p{p}", name=f"s1p{p}")
                   for p in range(PAIRS)]
        Copy = mybir.ActivationFunctionType.Copy
        for ck in range(NCHUNK):
            c0 = ck * cols
            xts = xts_list[ck]
            if ck >= 3:
                nc.sync.dma_start(xts[:], xT_d[:, :, :, c0:c0 + cols])
            # wT slices for this chunk (fp8 DoubleRow)
            for di in range(CH):
                wps = psB.tile([128, 512], F32, tag="psB", name="wps")
                for g in range(2):
                    nc.tensor.matmul(
                        wps[:], pwT8[:, g, :, di * 128:(di + 1) * 128],
                        xts[:, g, :, :],
                        start=(g == 0), stop=(g == 1),
                        perf_mode=mybir.MatmulPerfMode.DoubleRow)
                dst = wtb[di][:, c0:c0 + cols]
                if di % 2 == 0:
                    nc.vector.tensor_scalar_mul(dst, wps[:], 1.0 / 16.0)
                else:
                    nc.scalar.activation(dst, wps[:], Copy,
                                         scale=1.0 / 16.0)
            # dots + exp for this chunk's 512-slice, all pairs
            for p in range(PAIRS):
                sl = ck
                dps = psB.tile([128, 512], F32, tag="psB", name="dps")
                nc.tensor.matmul(dps[:], dblk[p][:],
                                 wtb[p][:, sl * 512:(sl + 1) * 512],
                                 start=True, stop=True)
                nc.scalar.activation(
                    ed_grp[p // 2][:, p % 2, sl * 512:(sl + 1) * 512],
                    dps[:], Exp,
                    scale=SCALE / 16.0,
                    accum_out=s1parts[p][:, sl:sl + 1])
            # w natural transposes for this chunk (4 token tiles)
            for t0 in range(ck * 4, ck * 4 + 4, 4):
                for di in range(CH):
                    wtp = psC.tile([128, 4, 256], FP8, tag="psC", name="wtp")
                    for j in range(4):
                        nc.tensor.transpose(
                            wtp[:, j, 0:256:2],
                            wtb[di][:, (t0 + j) * 128:(t0 + j + 1) * 128],
                            ident8[:])
                    if di % 2 == 0:
                        nc.vector.tensor_copy(
                            w_sb[:, t0:t0 + 4, di * 128:(di + 1) * 128],
                            wtp[:, :, 0:256:2])
                    else:
                        nc.scalar.copy(
                            w_sb[:, t0:t0 + 4, di * 128:(di + 1) * 128],
                            wtp[:, :, 0:256:2])
            # attn transposes for this chunk
            for p in range(PAIRS):
                for t0 in range(ck * 4, ck * 4 + 4, 4):
                    atp = psC.tile([128, 4, 256], FP8, tag="psC", name="atp")
                    for j in range(4):
                        nc.tensor.transpose(
                            atp[:, j, 0:256:2],
                            ed_grp[p // 2][:, p % 2,
                                           (t0 + j) * 128:(t0 + j + 1) * 128],
                            ident8[:])
                    nc.vector.tensor_copy(at_list[p][:, t0:t0 + 4, :],
                                          atp[:, :, 0:256:2])

        # ============ P4: pooled attention tail (pairs interleaved) ======
        ssc_l, rnat_l, rnT_l, ed2_l, xds_l, xdsT_l = [], [], [], [], [], []
        for p in range(PAIRS):
            s1 = sm_pool.tile([128, 1], F32, tag=f"s1_{p}", name=f"s1_{p}")
            nc.vector.tensor_reduce(s1[:], s1parts[p][:], X, ADD)
            rc1 = sm_pool.tile([128, 1], F32, tag=f"rc1_{p}", name=f"rc1_{p}")
            nc.vector.reciprocal(rc1[:], s1[:])
            ssc = sm_pool.tile([128, 1], F32, tag=f"ssc_{p}", name=f"ssc_{p}")
            nc.vector.tensor_mul(ssc[:], rc1[:], srep[:, p:p + 1])
            ssc_l.append((rc1, ssc))
        rdsb_l = []
        for p in range(PAIRS):
            rd_ps = psD.tile([128, 128], F32, tag="psD", name=f"rd{p}")
            for t in range(NT):
                nc.tensor.matmul(rd_ps[:], at_list[p][:, t, :],
                                 w_sb[:, t, p * 128:(p + 1) * 128],
                                 start=(t == 0), stop=(t == NT - 1))
            rdsb = sm_pool.tile([128, 128], F32, tag=f"rdsb{p}",
                                name=f"rdsb{p}")
            nc.vector.tensor_copy(rdsb[:], rd_ps[:])
            rdsb_l.append(rdsb)
        for p in range(PAIRS):
            rnat = sm_pool.tile([128, 128], BF16, tag=f"rnat{p}",
                                name=f"rnat{p}")
            nc.vector.memset(rnat[:], 0.0)
            for h in range(2):
                r0, r1 = 64 * h, 64 * (h + 1)
                nc.vector.scalar_tensor_tensor(
                    rnat[r0:r1, r0:r1], rdsb_l[p][r0:r1, r0:r1],
                    ssc_l[p][1][r0:r1, 0:1],
                    rpT[p][r0:r1, r0:r1], MULT, ADD)
            rnat_l.append(rnat)
        for p in range(PAIRS):
            rtp = psC.tile([128, 128], BF16, tag="psC", name=f"rtp{p}")
            nc.tensor.transpose(rtp[:], rnat_l[p][:], identb[:])
            rnT = sm_pool.tile([128, 128], BF16, tag=f"rnT{p}", name=f"rnT{p}")
            nc.vector.tensor_copy(rnT[:], rtp[:])
            rnT_l.append(rnT)
        s2_l = []
        for p in range(PAIRS):
            d2_ps = psD.tile([128, 128], F32, tag="psD", name=f"d2{p}")
            nc.tensor.matmul(d2_ps[:], rnT_l[p][:], rnT_l[p][:],
                             start=True, stop=True)
            ed2 = sm_pool.tile([128, 128], BF16, tag=f"ed2_{p}",
                               name=f"ed2_{p}")
            nc.vector.memset(ed2[:], 0.0)
            s2 = sm_pool.tile([128, 1], F32, tag=f"s2_{p}", name=f"s2_{p}")
            for h in range(2):
                r0, r1 = 64 * h, 64 * (h + 1)
                nc.scalar.activation(ed2[r0:r1, r0:r1], d2_ps[r0:r1, r0:r1],
                                     Exp, scale=SCALE,
                                     accum_out=s2[r0:r1, 0:1])
            ed2_l.append(ed2)
            s2_l.append(s2)
        for p in range(PAIRS):
            xds_ps = psD.tile([128, 128], F32, tag="psD", name=f"xds{p}")
            nc.tensor.matmul(xds_ps[:], ed2_l[p][:], rnat_l[p][:],
                             start=True, stop=True)
            rc2 = sm_pool.tile([128, 1], F32, tag=f"rc2_{p}", name=f"rc2_{p}")
            nc.vector.reciprocal(rc2[:], s2_l[p][:])
            sc = sm_pool.tile([128, 1], F32, tag=f"sc_{p}", name=f"sc_{p}")
            nc.vector.tensor_mul(sc[:], ssc_l[p][0][:], rc2[:])
            xds = sm_pool.tile([128, 128], BF16, tag=f"xds_{p}",
                               name=f"xds_{p}")
            nc.vector.tensor_scalar_mul(xds[:], xds_ps[:], sc[:])
            xds_l.append(xds)
        for p in range(PAIRS):
            xtp = psC.tile([128, 128], BF16, tag="psC", name=f"xtp{p}")
            nc.tensor.transpose(xtp[:], xds_l[p][:], identb[:])
            xdsT = sm_pool.tile([128, 128], BF16, tag=f"xdsT{p}",
                                name=f"xdsT{p}")
            nc.vector.tensor_copy(xdsT[:], xtp[:])
            xdsT_l.append(xdsT)
        for p in range(PAIRS):
            mp_ps = psB.tile([128, 512], F32, tag="psB", name=f"mp_ps{p}")
            nc.tensor.matmul(mp_ps[:], xdsT_l[p][:], twT[:, p, :],
                             start=True, stop=True)
            nc.vector.tensor_scalar_mul(M_grp[p // 2][:, p % 2, :], mp_ps[:],
                                        16.0)

    # ============ P5: outT[d, n] = sum_p M_p^T @ ed_p  (+bias) ==========
    with tc.tile_pool(name="psA", bufs=3, space="PSUM") as psA:
        for dc in range(CH):
            osb = ost_pool.tile([128, N], BF16, tag="osb", name="osb")
            for g in range(NS // 2):
                ops = psA.tile([128, 2, 512], F32, tag="psA", name="ops")
                for sli in range(2):
                    sl = g * 2 + sli
                    for u in range(PAIRS // 2):
                        nc.tensor.matmul(
                            ops[:, sli, :],
                            M_grp[u][:, :, dc * 128:(dc + 1) * 128],
                            ed_grp[u][:, :, sl * 512:(sl + 1) * 512],
                            start=(u == 0), stop=(u == PAIRS // 2 - 1),
                            perf_mode=mybir.MatmulPerfMode.DoubleRow)
                dst = osb[:, g * 1024:(g + 1) * 1024]
                osrc = ops.rearrange("p a b -> p (a b)")
                if g % 2 == 0:
                    nc.scalar.activation(dst, osrc[:], Ident,
                                         bias=biasT[:, dc:dc + 1],
                                         scale=1.0 / 16.0)
                else:
                    nc.vector.tensor_scalar(dst, osrc[:], 1.0 / 16.0,
                                            biasT[:, dc:dc + 1], MULT, ADD)
                nc.sync.dma_start(
                    out_d[dc * 128:(dc + 1) * 128, g * 1024:(g + 1) * 1024],
                    osb[:, g * 1024:(g + 1) * 1024])


def _prep_inputs(x, proj_w, step_rep, step_x, to_out_w, to_out_b):
    x = np.asarray(x, dtype=np.float32)
    proj_w = np.asarray(proj_w, dtype=np.float32)
    step_rep = np.asarray(step_rep, dtype=np.float32).reshape(HEADS)
    step_x = np.asarray(step_x, dtype=np.float32).reshape(HEADS)
    to_out_w = np.asarray(to_out_w, dtype=np.float32)
    to_out_b = np.asarray(to_out_b, dtype=np.float32)

    pw16 = (proj_w.T * 16.0).reshape(2, 128, 2, C)
    pwT8 = np.ascontiguousarray(pw16.transpose(1, 0, 2, 3)).astype(
        ml_dtypes.float8_e4m3)
    twTs = np.ascontiguousarray(to_out_w.T) * np.repeat(step_x, DH)[:, None]
    twTs = np.ascontiguousarray(
        twTs.reshape(CH, 128, C).transpose(1, 0, 2)).astype(ml_dtypes.bfloat16)
    biasT = np.ascontiguousarray(
        to_out_b.reshape(CH, 128).T).astype(np.float32)

    srep = np.empty((128, PAIRS), dtype=np.float32)
    for p in range(PAIRS):
        srep[0:64, p] = step_rep[2 * p]
        srep[64:128, p] = step_rep[2 * p + 1]

    identb = np.eye(128, dtype=ml_dtypes.bfloat16)
    ident8 = np.eye(128, dtype=ml_dtypes.float8_e4m3)

    shared = {
        "pwT8": pwT8, "twT": twTs, "biasT": biasT,
        "srep": srep, "identb": identb, "ident8": ident8,
    }
    in_maps = []
    for b in range(B):
        xT = np.ascontiguousarray(
            x[b].T.reshape(2, 128, 2, N).transpose(1, 0, 2, 3)).astype(
            ml_dtypes.float8_e4m3)
        # pooled x: spatial 64x64 grid, 8x8 block means -> [64, C]
        xp = x[b].reshape(8, 8, 8, 8, C).mean(axis=(1, 3)).reshape(Q, C)
        xpT = np.ascontiguousarray(
            (xp.T * 16.0).reshape(2, 128, 2, Q).transpose(1, 0, 2, 3)).astype(
            ml_dtypes.float8_e4m3)
        in_maps.append({"xT": xT, "xpT": xpT, **shared})
    return in_maps


def kernel(x, proj_w, step_rep, step_x, to_out_w, to_out_b):
    if "nc" not in _CACHE:
        _CACHE["nc"] = _build()
    nc = _CACHE["nc"]
    in_maps = _prep_inputs(x, proj_w, step_rep, step_x, to_out_w, to_out_b)
    res = bass_utils.run_bass_kernel_spmd(nc, in_maps, core_ids=list(range(B)))
    return np.stack(
        [np.asarray(res.results[b]["out"]).astype(np.float32).T
         for b in range(B)], axis=0)


# revision 28
# speedup vs baseline: 1.9103x; 1.1248x over previous
"""Trainium2 Bass kernel for CBSA (cross-block self-attention) module.

Shapes (hardcoded from the problem spec):
  x: [8, 4096, 512], proj_w/to_out_w: [512, 512], step_rep/step_x: [8,1,1],
  to_out_b: [512].  Output: [8, 4096, 512].

Sharding: data-parallel over batch, 1 batch per NeuronCore (8 cores).

Structure:
  - pooling is linear and commutes with the proj GEMM, so pooled x is
    computed on host and rep^T comes from a tiny fp8 on-device GEMM.
  - P1 streams x^T (fp8) in 8 chunks; per chunk: fp8-DoubleRow wT GEMM,
    dots (block-diag rep lhsT), exp -> ed (fp8), and packed transposes:
    a bf16 [128,128] transpose moves a PAIR of adjacent fp8 n-columns,
    halving transpose count; copies run at 2-byte DVE speed.
  - rep_delta contracts the packed pairs via two stride-2 byte-plane fp8
    matmuls per tile; an extra fp8(1,1) column in the packed w tiles
    makes the same matmuls accumulate the softmax row-sums s1 for free.
  - per-pair tail (pairs interleaved step-wise to keep PE fed):
    rep update, pooled self-attention, xds^T computed directly as
    rnat^T @ ed2 (ed2 is symmetric), M = xds^T' @ to_out^T with the
    1/(s1*s2) scale folded into the M copy.
  - P5: out^T[d, n] accumulated over pair-groups with fp8-DoubleRow
    matmuls; bias is a per-partition scalar fused into the PSUM->SBUF
    copy; output is bf16 [C, N], host transposes/casts back.
"""

from contextlib import ExitStack

import numpy as np
import ml_dtypes

import concourse.bass as bass
import concourse.tile as tile
from concourse import bacc, mybir
from concourse import bass_utils

F32 = mybir.dt.float32
BF16 = mybir.dt.bfloat16
FP8 = mybir.dt.float8e4

B = 8
N = 4096
C = 512
HEADS = 8
DH = 64
Q = 64            # pooled tokens
SCALE = DH ** -0.5
NT = N // 128     # 32 token tiles
CH = C // 128     # 4 feature chunks
PAIRS = HEADS // 2  # 4 head pairs
NS = N // 512     # 8 free-dim slices of 512
NCHUNK = 8        # x streamed in 8 chunks of 512 tokens

_CACHE = {}


def _build():
    nc = bacc.Bacc("TRN2", target_bir_lowering=False, debug=False, num_devices=B)

    xT_d = nc.dram_tensor("xT", [128, 2, 2, N], FP8, kind="ExternalInput").ap()
    pwT8_d = nc.dram_tensor("pwT8", [128, 2, 2, C], FP8, kind="ExternalInput").ap()
    dblk_d = nc.dram_tensor("dblk8", [128, PAIRS, 128], FP8,
                            kind="ExternalInput").ap()
    rpT_d = nc.dram_tensor("rpTb", [128, PAIRS, 128], BF16,
                           kind="ExternalInput").ap()
    twT_d = nc.dram_tensor("twT", [128, CH, C], BF16, kind="ExternalInput").ap()
    biasT_d = nc.dram_tensor("biasT", [128, CH], F32, kind="ExternalInput").ap()
    srep_d = nc.dram_tensor("srep", [128, PAIRS], F32, kind="ExternalInput").ap()
    idb_d = nc.dram_tensor("identb", [128, 128], BF16, kind="ExternalInput").ap()
    out_d = nc.dram_tensor("out", [C, N], BF16, kind="ExternalOutput").ap()

    with tile.TileContext(nc) as tc:
        with ExitStack() as ctx:
            _body.ctx = ctx
            _body(tc, nc, xT_d, pwT8_d, dblk_d, rpT_d, twT_d, biasT_d,
                  srep_d, idb_d, out_d)
    nc.compile()
    return nc


def _body(tc, nc, xT_d, pwT8_d, dblk_d, rpT_d, twT_d, biasT_d, srep_d,
          idb_d, out_d):
    Exp = mybir.ActivationFunctionType.Exp
    Ident = mybir.ActivationFunctionType.Identity
    X = mybir.AxisListType.X
    ADD = mybir.AluOpType.add
    MULT = mybir.AluOpType.mult

    ctx = _body.ctx
    const = ctx.enter_context(tc.tile_pool(name="const", bufs=1))
    persist = ctx.enter_context(tc.tile_pool(name="persist", bufs=1))
    xs_pool = ctx.enter_context(tc.tile_pool(name="xstream", bufs=2))
    sm_pool = ctx.enter_context(tc.tile_pool(name="small", bufs=2))
    ost_pool = ctx.enter_context(tc.tile_pool(name="ostage", bufs=2))

    # ---- constants / small inputs (issued before the big x stream) ----
    pwT8 = const.tile([128, 2, 2, C], FP8, tag="pwT8")
    nc.sync.dma_start(pwT8[:], pwT8_d[:])
    dblk_t = const.tile([128, PAIRS, 128], FP8, tag="dblk_t")
    nc.sync.dma_start(dblk_t[:], dblk_d[:])
    rpT_t = const.tile([128, PAIRS, 128], BF16, tag="rpT_t")
    nc.sync.dma_start(rpT_t[:], rpT_d[:])
    # prefetch first two x chunks ahead of the remaining constants
    cols = N // NCHUNK
    xts_list = []
    for ck in range(NCHUNK):
        xts = xs_pool.tile([128, 2, 2, cols], FP8, tag="xs", name=f"xts{ck}")
        xts_list.append(xts)
    for ck in range(3):
        nc.sync.dma_start(xts_list[ck][:],
                          xT_d[:, :, :, ck * cols:(ck + 1) * cols])
    identb = const.tile([128, 128], BF16, tag="identb")
    nc.sync.dma_start(identb[:], idb_d[:])
    srep = const.tile([128, PAIRS], F32, tag="srep")
    nc.sync.dma_start(srep[:], srep_d[:])
    twT = const.tile([128, CH, C], BF16, tag="twT")
    nc.sync.dma_start(twT[:], twT_d[:])
    biasT = const.tile([128, CH], F32, tag="biasT")
    nc.sync.dma_start(biasT[:], biasT_d[:])

    # ---- persistent intermediates ----
    wtb = [persist.tile([128, N], FP8, tag=f"wtb{di}", name=f"wtb{di}")
           for di in range(CH)]
    # packed w natural: bf16 element (n2, d) = fp8 pair (w[2*n2,d], w[2*n2+1,d])
    # per (tile, pair) block of 129 cols: [0:128] w-pairs, col 128 = fp8(1,1)
    # so rep_delta's DR matmul also accumulates s1 = sum_n at[n, q] in col 128.
    w_sb = persist.tile([128, NT // 2, PAIRS, 129], BF16, tag="w_sb")
    ed_grp = [persist.tile([128, 2, N], FP8, tag=f"edg{u}", name=f"edg{u}")
              for u in range(PAIRS // 2)]
    # packed attn^T: bf16 element (n2, q) = fp8 pair (ed[q,2*n2], ed[q,2*n2+1])
    at_list = [persist.tile([128, NT // 2, 128], BF16, tag=f"at{p}",
                            name=f"at{p}")
               for p in range(PAIRS)]
    M_grp = [persist.tile([128, 2, C], FP8, tag=f"Mg{u}", name=f"Mg{u}")
             for u in range(PAIRS // 2)]

    with ExitStack() as phase_ctx:
        psB = phase_ctx.enter_context(
            tc.tile_pool(name="psB", bufs=3, space="PSUM"))
        psC = phase_ctx.enter_context(
            tc.tile_pool(name="psC", bufs=2, space="PSUM"))
        psD = phase_ctx.enter_context(
            tc.tile_pool(name="psD", bufs=3, space="PSUM"))

        ONES_PAIR = float(np.frombuffer(bytes([0x38, 0x38]),
                                        ml_dtypes.bfloat16)[0])
        nc.vector.memset(w_sb[:, :, :, 128:129], ONES_PAIR)

        # rep (pooled queries) is computed on host: dblk = 16*repT
        # block-diag per pair (fp8, dots lhsT), rpT = its natural-layout
        # transpose (bf16, rep update operand).
        dblk = [dblk_t[:, p, :] for p in range(PAIRS)]
        rpT = [rpT_t[:, p, :] for p in range(PAIRS)]

        # ============ P1: streamed wT GEMM + dots + exp + transposes ======
        # wT[d, n] = sum_c pw[d, c] x[n, c]; lhsT = pwT chunk, rhs = xT chunk
        Copy = mybir.ActivationFunctionType.Copy
        for ck in range(NCHUNK):
            c0 = ck * cols
            xts = xts_list[ck]
            if ck >= 3:
                nc.sync.dma_start(xts[:], xT_d[:, :, :, c0:c0 + cols])
            # wT slices for this chunk (fp8 DoubleRow)
            for di in range(CH):
                wps = psB.tile([128, 512], F32, tag="psB", name="wps")
                for g in range(2):
                    nc.tensor.matmul(
                        wps[:], pwT8[:, g, :, di * 128:(di + 1) * 128],
                        xts[:, g, :, :],
                        start=(g == 0), stop=(g == 1),
                        perf_mode=mybir.MatmulPerfMode.DoubleRow)
                dst = wtb[di][:, c0:c0 + cols]
                nc.vector.tensor_scalar_mul(dst, wps[:], 1.0 / 16.0)
            # dots + exp for this chunk's 512-slice, all pairs
            for p in range(PAIRS):
                sl = ck
                dps = psB.tile([128, 512], F32, tag="psB", name="dps")
                nc.tensor.matmul(dps[:], dblk[p][:],
                                 wtb[p][:, sl * 512:(sl + 1) * 512],
                                 start=True, stop=True)
                nc.scalar.activation(
                    ed_grp[p // 2][:, p % 2, sl * 512:(sl + 1) * 512],
                    dps[:], Exp, scale=SCALE / 16.0)
            # packed transposes every 2 chunks: bf16 view pairs adjacent
            # fp8 n-columns, so one [128,128] bf16 transpose covers 2 tiles.
            if ck % 2 == 1:
                t4 = ck // 2            # 4 bf16 n2-tiles per 2-chunk group

                def w_tr(di):
                    wtp = psC.tile([128, 4, 128], BF16, tag="psC", name="wtp")
                    wvv = wtb[di].bitcast(BF16)
                    for j in range(4):
                        nc.tensor.transpose(
                            wtp[:, j, :],
                            wvv[:, (t4 * 4 + j) * 128:(t4 * 4 + j + 1) * 128],
                            identb[:])
                    nc.vector.tensor_copy(
                        w_sb[:, t4 * 4:t4 * 4 + 4, di, 0:128], wtp[:])

                def at_tr(p):
                    atp = psC.tile([128, 4, 128], BF16, tag="psC", name="atp")
                    evv = ed_grp[p // 2][:, p % 2, :].bitcast(BF16)
                    for j in range(4):
                        nc.tensor.transpose(
                            atp[:, j, :],
                            evv[:, (t4 * 4 + j) * 128:(t4 * 4 + j + 1) * 128],
                            identb[:])
                    nc.vector.tensor_copy(
                        at_list[p][:, t4 * 4:t4 * 4 + 4, :], atp[:])

                if ck == NCHUNK - 1:
                    # final chunk: interleave so rep_delta(p) unblocks
                    # progressively in pair order
                    for p in range(PAIRS):
                        w_tr(p)
                        at_tr(p)
                else:
                    for di in range(CH):
                        w_tr(di)
                    for p in range(PAIRS):
                        at_tr(p)

        # ============ P4: pooled attention tail (pairs interleaved) ======
        ssc_l, rnat_l, rnT_l, ed2_l, xds_l, xdsT_l = [], [], [], [], [], []
        rdsb_l = []
        for p in range(PAIRS):
            rd_ps = psD.tile([128, 129], F32, tag="psD", name=f"rd{p}")
            for t in range(NT // 2):
                atv = at_list[p][:, t, :].bitcast(FP8)
                wvv = w_sb[:, t, p, :].bitcast(FP8)
                for j in range(2):
                    nc.tensor.matmul(rd_ps[:], atv[:, j:256:2],
                                     wvv[:, j:258:2],
                                     start=(t == 0 and j == 0),
                                     stop=(t == NT // 2 - 1 and j == 1))
            rdsb = sm_pool.tile([128, 129], F32, tag=f"rdsb{p}",
                                name=f"rdsb{p}")
            nc.vector.tensor_copy(rdsb[:], rd_ps[:])
            rdsb_l.append(rdsb)
        for p in range(PAIRS):
            rc1 = sm_pool.tile([128, 1], F32, tag=f"rc1_{p}", name=f"rc1_{p}")
            nc.vector.reciprocal(rc1[:], rdsb_l[p][:, 128:129])
            ssc = sm_pool.tile([128, 1], F32, tag=f"ssc_{p}", name=f"ssc_{p}")
            nc.vector.tensor_mul(ssc[:], rc1[:], srep[:, p:p + 1])
            ssc_l.append((rc1, ssc))
        for p in range(PAIRS):
            rnat = sm_pool.tile([128, 128], BF16, tag=f"rnat{p}",
                                name=f"rnat{p}")
            nc.vector.memset(rnat[:], 0.0)
            for h in range(2):
                r0, r1 = 64 * h, 64 * (h + 1)
                nc.vector.scalar_tensor_tensor(
                    rnat[r0:r1, r0:r1], rdsb_l[p][r0:r1, r0:r1],
                    ssc_l[p][1][r0:r1, 0:1],
                    rpT[p][r0:r1, r0:r1], MULT, ADD)
            rnat_l.append(rnat)
        for p in range(PAIRS):
            rtp = psC.tile([128, 128], BF16, tag="psC", name=f"rtp{p}")
            nc.tensor.transpose(rtp[:], rnat_l[p][:], identb[:])
            rnT = sm_pool.tile([128, 128], BF16, tag=f"rnT{p}", name=f"rnT{p}")
            nc.vector.tensor_copy(rnT[:], rtp[:])
            rnT_l.append(rnT)
        s2_l = []
        for p in range(PAIRS):
            d2_ps = psD.tile([128, 128], F32, tag="psD", name=f"d2{p}")
            nc.tensor.matmul(d2_ps[:], rnT_l[p][:], rnT_l[p][:],
                             start=True, stop=True)
            ed2 = sm_pool.tile([128, 128], BF16, tag=f"ed2_{p}",
                               name=f"ed2_{p}")
            nc.vector.memset(ed2[:], 0.0)
            s2 = sm_pool.tile([128, 1], F32, tag=f"s2_{p}", name=f"s2_{p}")
            for h in range(2):
                r0, r1 = 64 * h, 64 * (h + 1)
                nc.scalar.activation(ed2[r0:r1, r0:r1], d2_ps[r0:r1, r0:r1],
                                     Exp, scale=SCALE,
                                     accum_out=s2[r0:r1, 0:1])
            ed2_l.append(ed2)
            s2_l.append(s2)
        sc_l = []
        for p in range(PAIRS):
            # xds^T[d, q] = sum_k rnat[k, d] * ed2[q, k]  (ed2 symmetric)
            xt_ps = psD.tile([128, 128], F32, tag="psD", name=f"xt{p}")
            nc.tensor.matmul(xt_ps[:], rnat_l[p][:], ed2_l[p][:],
                             start=True, stop=True)
            xdsT = sm_pool.tile([128, 128], BF16, tag=f"xdsT{p}",
                                name=f"xdsT{p}")
            nc.vector.tensor_copy(xdsT[:], xt_ps[:])
            xdsT_l.append(xdsT)
            rc2 = sm_pool.tile([128, 1], F32, tag=f"rc2_{p}", name=f"rc2_{p}")
            nc.vector.reciprocal(rc2[:], s2_l[p][:])
            sc = sm_pool.tile([128, 1], F32, tag=f"sc_{p}", name=f"sc_{p}")
            nc.vector.tensor_mul(sc[:], ssc_l[p][0][:], rc2[:])
            sc_l.append(sc)
        for p in range(PAIRS):
            mp_ps = psB.tile([128, 512], F32, tag="psB", name=f"mp_ps{p}")
            nc.tensor.matmul(mp_ps[:], xdsT_l[p][:], twT[:, p, :],
                             start=True, stop=True)
            nc.vector.tensor_scalar(M_grp[p // 2][:, p % 2, :], mp_ps[:],
                                    sc_l[p][:], 16.0, MULT, MULT)

    # ============ P5: outT[d, n] = sum_p M_p^T @ ed_p  (+bias) ==========
    with tc.tile_pool(name="psA", bufs=6, space="PSUM") as psA:
        for dc in range(CH):
            osb = ost_pool.tile([128, N], BF16, tag="osb", name="osb")
            for sl in range(NS):
                ops = psA.tile([128, 512], F32, tag="psA", name="ops")
                for u in range(PAIRS // 2):
                    nc.tensor.matmul(
                        ops[:],
                        M_grp[u][:, :, dc * 128:(dc + 1) * 128],
                        ed_grp[u][:, :, sl * 512:(sl + 1) * 512],
                        start=(u == 0), stop=(u == PAIRS // 2 - 1),
                        perf_mode=mybir.MatmulPerfMode.DoubleRow)
                dst = osb[:, sl * 512:(sl + 1) * 512]
                if sl % 2 == 0:
                    nc.scalar.activation(dst, ops[:], Ident,
                                         bias=biasT[:, dc:dc + 1],
                                         scale=1.0 / 16.0)
                else:
                    nc.vector.tensor_scalar(dst, ops[:], 1.0 / 16.0,
                                            biasT[:, dc:dc + 1], MULT, ADD)
                if sl % 2 == 1:
                    nc.sync.dma_start(
                        out_d[dc * 128:(dc + 1) * 128,
                              (sl - 1) * 512:(sl + 1) * 512],
                        osb[:, (sl - 1) * 512:(sl + 1) * 512])
